# revision 6
# baseline (speedup 1.0000x reference)
"""GQA attention kernel for Trainium2, tensor-parallel over heads on 8 cores.

Problem: B=1, T=2048, EMB=4096, H=32 query heads, G=8 KV groups, D=128.
Reference: q/k/v projections -> per-head RMS norm (q,k) -> RoPE (q,k) ->
causal GQA attention -> out projection.

Sharding: core c owns query heads [4c, 4c+4) and KV group c.  Each core
computes a partial output x @ W_o_rows for its heads; host sums the 8
partials (the all-reduce of the module's TP scheme).

Design notes:
  - softmax denominator is FREE: ctx matmuls use P-block [k,128q] as the
    stationary operand and [v | ones] (129 moving cols) as the moving one;
    den accumulates in psum col 128 of the same accumulation group.
  - normalization is a per-partition tensor_scalar multiply (q is the
    partition dim of the ctx psum tile) -- no partition broadcast needed.
  - attention runs in 256-query slices; causal sub-block skipping: the
    ctx matmul (qb, jb) is only issued when jb <= 2*si+qb, so the
    fully-masked P regions are never read; the mask is a [128,128]
    triangle added only on diagonal subblocks (post-matmul on DVE --
    PE accumulate does NOT see DVE-prewritten psum on real HW).
  - qkv biases fold into the projection accumulation as a K=1 matmul
    (ones[1,128] x brow[1,768]); when the biases are all zero (the
    common case) a specialized program variant skips them entirely.
  - each open psum accumulation group needs its own bank (zero region):
    8 banks = psS(2)+psS2(1) S-groups, ctxq0(1), ctxq1(1), psT2(1),
    psO(2); S-groups rotate 3-deep across psS/psS2.
  - phases: A (projections+rms+rope+transpose), then B (attention) and
    C (out projection) fused: C ot-groups are drained into the B stream
    (2 per group-iteration) so PE fills ACT-bound exp slack; the first
    B S/exp groups are issued during the last A tile.
  - the SP queue issues DMAs serially (blocking for the transfer), so
    DMA program order == arrival order: wqkv e0-3, x strip 0, e4-15,
    x strip 1, rope tables, consts, remaining wqkv; wo/mask issued at
    A-tile 2; exp table loaded once (Square/Copy live in every set).
    The last A tile copies q/kv psum to SBUF so the A psum banks (and
    with them the B pools) free before its rms/rope chain drains.
"""

import numpy as np
import ml_dtypes
from contextlib import ExitStack

import concourse.bacc as bacc
import concourse.mybir as mybir
from concourse.tile import TileContext
from concourse.bass_utils import run_bass_kernel_spmd

EMB, H, G, D, T = 4096, 32, 8, 128, 2048
EPS = 1e-6
NCORES = 8
HP = H // NCORES          # 4 query heads per core
NT = T // 128             # 16 t-tiles
NE = EMB // 128           # 32 e-tiles
QW = HP * D               # 512 = q width per core
KVW = 2 * D               # 256 = k|v width per core
WW = QW + KVW             # 768 = packed qkv width per core
SM_SCALE = 1.0 / float(np.sqrt(D))
NEG = -1e9
OC = 512                  # phase C output column tile width
NO = EMB // OC            # 16 output column tiles

F32 = mybir.dt.float32
BF16 = mybir.dt.bfloat16
BF = ml_dtypes.bfloat16

_prog_cache = {}


def _build_program(zero_bias):
    nc = bacc.Bacc()

    xT_d = nc.declare_dram_parameter("xT", [NT * 128, NE * 128], BF16, isOutput=False)
    wqkv_d = nc.declare_dram_parameter("wqkv", [NE * 128, WW], BF16, isOutput=False)
    wo_d = nc.declare_dram_parameter("wo", [HP * 128, EMB], BF16, isOutput=False)
    cq_d = nc.declare_dram_parameter("cosq", [NT * 128, 128], F32, isOutput=False)
    sq_d = nc.declare_dram_parameter("sinq", [NT * 128, 128], F32, isOutput=False)
    ck_d = nc.declare_dram_parameter("cosk", [NT * 128, 128], F32, isOutput=False)
    sk_d = nc.declare_dram_parameter("sink", [NT * 128, 128], F32, isOutput=False)
    mask_d = nc.declare_dram_parameter("mask256", [128, 512], F32, isOutput=False)
    ident_d = nc.declare_dram_parameter("ident", [128, 128], BF16, isOutput=False)
    brow_d = (None if zero_bias else
              nc.declare_dram_parameter("brow", [1, WW], BF16, isOutput=False))
    out_d = nc.declare_dram_parameter("out", [T, EMB], F32, isOutput=True)

    with TileContext(nc) as tc, ExitStack() as ctx:
        consts = ctx.enter_context(tc.tile_pool(name="consts", bufs=1))
        wpool = ctx.enter_context(tc.tile_pool(name="wpool", bufs=1))
        xpool = ctx.enter_context(tc.tile_pool(name="xpool", bufs=2))
        cspool = ctx.enter_context(tc.tile_pool(name="cspool", bufs=2))
        scratch = ctx.enter_context(tc.tile_pool(name="scratch", bufs=8))
        small = ctx.enter_context(tc.tile_pool(name="small", bufs=12))
        resid = ctx.enter_context(tc.tile_pool(name="resid", bufs=1))
        ppool = ctx.enter_context(tc.tile_pool(name="ppool", bufs=6))
        opool = ctx.enter_context(tc.tile_pool(name="opool", bufs=6))

        # ---- early DMAs: the SP queue issues serially, so order = arrival.
        # First-needed first: wqkv e=0-3, x strip 0, more wqkv, x strip 1,
        # rope tables for tiles 0-1, brow/ident, remaining wqkv strips.
        TILE_ORDER = list(range(NT))
        wqkv_sb = [wpool.tile([128, WW], BF16, tag=f"wqkv{e}", name=f"wqkv{e}")
                   for e in range(NE)]

        def _wq_dma(e):
            nc.sync.dma_start(out=wqkv_sb[e],
                              in_=wqkv_d[e * 128:(e + 1) * 128, :])

        for e in range(4):
            _wq_dma(e)
        xs_hoist = {}
        xs = xpool.tile([128, NE * 128], BF16, tag="xstrip", name="xstrip")
        nc.sync.dma_start(out=xs, in_=xT_d[0:128, :])
        xs_hoist[0] = xs
        for e in range(4, 11):
            _wq_dma(e)
        xs = xpool.tile([128, NE * 128], BF16, tag="xstrip", name="xstrip")
        nc.sync.dma_start(out=xs, in_=xT_d[128:256, :])
        xs_hoist[1] = xs
        cs_hoist = {}
        for it in TILE_ORDER[:2]:
            tiles = []
            for nm, d in (("cq", cq_d), ("sq", sq_d), ("ck", ck_d), ("sk", sk_d)):
                t = cspool.tile([128, 128], F32, tag=nm, name=nm)
                nc.sync.dma_start(out=t, in_=d[it * 128:(it + 1) * 128, :])
                tiles.append(t)
            cs_hoist[it] = tiles
        ident = consts.tile([128, 128], BF16, tag="ident", name="ident")
        nc.sync.dma_start(out=ident, in_=ident_d[:, :])
        eps_t = consts.tile([128, 1], F32, tag="eps", name="eps")
        nc.vector.memset(eps_t, EPS)
        if not zero_bias:
            brow = consts.tile([1, WW], BF16, tag="brow", name="brow")
            nc.sync.dma_start(out=brow, in_=brow_d[:, :])
            ones1 = consts.tile([1, 128], BF16, tag="ones1", name="ones1")
            nc.vector.memset(ones1, 1.0)
        for e in range(11, NE):
            _wq_dma(e)
        mask_sb = consts.tile([128, 512], F32, tag="mask", name="mask")
        wo_sb = [wpool.tile([128, EMB], BF16, tag=f"wo{h}", name=f"wo{h}")
                 for h in range(HP)]

        # resident activations
        qT = [resid.tile([128, T], BF16, tag=f"qT{h}", name=f"qT{h}") for h in range(HP)]
        kT = resid.tile([128, T], BF16, tag="kT", name="kT")
        vone = [resid.tile([128, 129], BF16, tag=f"v{j}", name=f"v{j}") for j in range(NT)]
        ctxT = [resid.tile([128, T], BF16, tag=f"ctxT{h}", name=f"ctxT{h}") for h in range(HP)]
        for j in range(NT):
            nc.vector.memset(vone[j][:, 128:129], 1.0)

        # ---------------- Phases ------------------------------------------
        # A: projections + rms + rope + transpose.  B: attention (256-query
        # slices, P-stationary ctx matmuls with a ones column for the free
        # softmax denominator).  C: out projection, interleaved into B's
        # stream so PE fills ACT-bound slack.  The first B groups are issued
        # during late phase A (psS opens alongside the A pools).
        psS_cm = tc.tile_pool(name="psS", bufs=2, space="PSUM")
        psA_cm = tc.tile_pool(name="psA", bufs=2, space="PSUM")
        psKV_cm = tc.tile_pool(name="psKV", bufs=2, space="PSUM")
        psT_cm = tc.tile_pool(name="psT", bufs=2, space="PSUM")
        psS = psS_cm.__enter__()
        psA = psA_cm.__enter__()
        psKV = psKV_cm.__enter__()
        psT = psT_cm.__enter__()

        NSI = 8
        glist = []
        for si in range(NSI):
            for h in range(HP):
                glist.extend((h, si, g) for g in range(si + 1))
        pend_p = {}
        ctx_tiles = {}
        psS2 = [None]

        def issue_s_mask_exp(idx):
            h, si, g = glist[idx]
            pool = psS if idx % 3 < 2 else psS2[0]
            s_g = pool.tile([128, 512], F32, tag="sg", name="sg")
            for half in range(2):
                jb = 2 * g + half
                nc.tensor.matmul(
                    s_g[:, half * 256:(half + 1) * 256],
                    kT[:, jb * 128:(jb + 1) * 128],
                    qT[h][:, si * 256:(si + 1) * 256],
                    start=True, stop=True,
                )
            for half in range(2):
                kk = 2 * g + half - 2 * si
                if kk >= 0:  # diagonal: triangular mask on its subblock
                    off = half * 256 + kk * 128
                    nc.vector.tensor_add(
                        s_g[:, off:off + 128], s_g[:, off:off + 128],
                        mask_sb[:, kk * 256 + kk * 128:kk * 256 + (kk + 1) * 128])
            p_g = ppool.tile([128, 512], BF16, tag="pt", name="pt")
            nc.scalar.activation(
                out=p_g, in_=s_g,
                func=mybir.ActivationFunctionType.Exp,
                scale=SM_SCALE,
            )
            pend_p[idx] = p_g

        LOOKAHEAD = 3

        def phase_a():
            for idx, it in enumerate(TILE_ORDER):
                q_ps = psA.tile([128, QW], F32, tag="aq", name="aq")
                kv_ps = psKV.tile([128, KVW], F32, tag="akv", name="akv")
                if it in xs_hoist:
                    xstrip = xs_hoist.pop(it)
                else:
                    xstrip = xpool.tile([128, NE * 128], BF16, tag="xstrip",
                                        name="xstrip")
                    nc.sync.dma_start(out=xstrip, in_=xT_d[it * 128:(it + 1) * 128, :])
                for e in range(NE):
                    xt = xstrip[:, e * 128:(e + 1) * 128]
                    last = zero_bias and e == NE - 1
                    nc.tensor.matmul(q_ps, xt, wqkv_sb[e][:, 0:QW],
                                     start=(e == 0), stop=last)
                    nc.tensor.matmul(kv_ps, xt, wqkv_sb[e][:, QW:WW],
                                     start=(e == 0), stop=last)
                if not zero_bias:
                    # bias add fused into the accumulation as a K=1 matmul
                    nc.tensor.matmul(q_ps, ones1, brow[:, 0:QW],
                                     start=False, stop=True)
                    nc.tensor.matmul(kv_ps, ones1, brow[:, QW:WW],
                                     start=False, stop=True)
                if idx == NT - 1:
                    # pre-fill the attention pipeline; si=0 needs only
                    # qT/kT t-tiles 0-1, ready long ago
                    issue_s_mask_exp(0)
                    issue_s_mask_exp(1)

                if it in cs_hoist:
                    cq, sq, ck, sk = cs_hoist.pop(it)
                else:
                    cq = cspool.tile([128, 128], F32, tag="cq", name="cq")
                    nc.sync.dma_start(out=cq, in_=cq_d[it * 128:(it + 1) * 128, :])
                    sq = cspool.tile([128, 128], F32, tag="sq", name="sq")
                    nc.sync.dma_start(out=sq, in_=sq_d[it * 128:(it + 1) * 128, :])
                    ck = cspool.tile([128, 128], F32, tag="ck", name="ck")
                    nc.sync.dma_start(out=ck, in_=ck_d[it * 128:(it + 1) * 128, :])
                    sk = cspool.tile([128, 128], F32, tag="sk", name="sk")
                    nc.sync.dma_start(out=sk, in_=sk_d[it * 128:(it + 1) * 128, :])
                if idx == 2:
                    nc.sync.dma_start(out=mask_sb, in_=mask_d[:, :])
                    for hh in range(HP):
                        nc.sync.dma_start(
                            out=wo_sb[hh], in_=wo_d[hh * 128:(hh + 1) * 128, :])

                if idx == NT - 1:
                    # free the A psum banks early: the B pools allocate them
                    qsb = scratch.tile([128, QW], F32, tag="qsb", name="qsb")
                    nc.scalar.copy(out=qsb, in_=q_ps)
                    kvsb = scratch.tile([128, KVW], F32, tag="kvsb", name="kvsb")
                    nc.scalar.copy(out=kvsb, in_=kv_ps)
                    q_src, kv_src = qsb, kvsb
                else:
                    q_src, kv_src = q_ps, kv_ps
                nc.vector.tensor_scalar_mul(
                    vone[it][:, 0:128], kv_src[:, 128:256], 1.0)
                for b in (HP, 0, 1, 2, 3):  # k first, then q heads
                    if b < HP:
                        src = q_src[:, b * 128:(b + 1) * 128]
                        c_t, s_t = cq[:, :], sq[:, :]
                    else:
                        src = kv_src[:, 0:128]
                        c_t, s_t = ck[:, :], sk[:, :]
                    sqout = scratch.tile([128, 128], F32, tag="sqout", name="sqout")
                    sqacc = small.tile([128, 1], F32, tag="sqacc", name="sqacc")
                    nc.scalar.activation(
                        out=sqout, in_=src,
                        func=mybir.ActivationFunctionType.Square,
                        accum_out=sqacc,
                    )
                    rstd = small.tile([128, 1], F32, tag="rstd", name="rstd")
                    nc.scalar.activation(
                        out=rstd, in_=sqacc,
                        func=mybir.ActivationFunctionType.Sqrt,
                        bias=eps_t, scale=1.0 / D,
                    )
                    nc.vector.reciprocal(rstd, rstd)
                    # rope: out1 = x1*c1 - x2*s1 ; out2 = x2*c2 + x1*s2
                    rt = scratch.tile([128, 128], F32, tag="rt", name="rt")
                    m1 = scratch.tile([128, 64], F32, tag="m1", name="m1")
                    nc.vector.tensor_mul(rt[:, 0:64], src[:, 0:64], c_t[:, 0:64])
                    nc.vector.tensor_mul(m1, src[:, 64:128], s_t[:, 0:64])
                    nc.vector.tensor_sub(rt[:, 0:64], rt[:, 0:64], m1)
                    m2 = scratch.tile([128, 64], F32, tag="m2", name="m2")
                    nc.vector.tensor_mul(rt[:, 64:128], src[:, 64:128], c_t[:, 64:128])
                    nc.vector.tensor_mul(m2, src[:, 0:64], s_t[:, 64:128])
                    nc.vector.tensor_add(rt[:, 64:128], rt[:, 64:128], m2)
                    rb = scratch.tile([128, 128], BF16, tag="rb", name="rb")
                    nc.vector.tensor_scalar_mul(rb, rt, rstd)
                    tp = psT.tile([128, 128], BF16, tag="tp", name="tp")
                    nc.tensor.transpose(tp, rb, ident)
                    dst = qT[b] if b < HP else kT
                    nc.scalar.copy(out=dst[:, it * 128:(it + 1) * 128], in_=tp)

        phase_a()
        psT_cm.__exit__(None, None, None)
        psKV_cm.__exit__(None, None, None)
        psA_cm.__exit__(None, None, None)

        # ---------------- Phases B+C fused, si-major ------------------------
        # Each open psum accumulation group needs its own bank (zero region):
        # ctx q-blocks live in separate single-buffered pools.
        with tc.tile_pool(name="psS2", bufs=1, space="PSUM") as psS2_pool, \
             tc.tile_pool(name="psC0", bufs=1, space="PSUM") as psC0, \
             tc.tile_pool(name="psC1", bufs=1, space="PSUM") as psC1, \
             tc.tile_pool(name="psT2", bufs=1, space="PSUM") as psT2, \
             tc.tile_pool(name="psO", bufs=2, space="PSUM") as psO:

            cwork = []
            psS2[0] = psS2_pool
            issue_s_mask_exp(2)

            def issue_ctx(idx):
                h, si, g = glist[idx]
                p_g = pend_p.pop(idx)
                if g == 0:
                    c0 = psC0.tile([128, 129], F32, tag="ctxq0", name="ctxq0")
                    c1 = psC1.tile([128, 129], F32, tag="ctxq1", name="ctxq1")
                    ctx_tiles[(h, si)] = (c0, c1)
                cts = ctx_tiles[(h, si)]
                for half in range(2):
                    jb = 2 * g + half
                    kk = jb - 2 * si
                    for qb in range(2):
                        if kk > qb:
                            continue  # fully-masked subblock: skip
                        nc.tensor.matmul(
                            cts[qb],
                            p_g[:, half * 256 + qb * 128:half * 256 + (qb + 1) * 128],
                            vone[jb],
                            start=(jb == 0), stop=(jb == 2 * si + qb),
                        )

            def normalize(h, si):
                cts = ctx_tiles.pop((h, si))
                for qb in range(2):
                    ct = cts[qb]
                    rden = small.tile([128, 1], F32, tag="rden", name="rden")
                    nc.vector.reciprocal(rden, ct[:, 128:129])
                    cbs = scratch.tile([128, 128], BF16, tag="cbs", name="cbs")
                    nc.vector.tensor_scalar_mul(cbs, ct[:, 0:128], rden)
                    tp = psT2.tile([128, 128], BF16, tag="tp2", name="tp2")
                    nc.tensor.transpose(tp, cbs, ident)
                    nc.vector.tensor_scalar_mul(
                        ctxT[h][:, si * 256 + qb * 128:si * 256 + (qb + 1) * 128],
                        tp, 1.0)

            def c_group(it, ot):
                o_ps = psO.tile([128, OC], F32, tag="ops", name="ops")
                for h in range(HP):
                    nc.tensor.matmul(
                        o_ps,
                        ctxT[h][:, it * 128:(it + 1) * 128],
                        wo_sb[h][:, ot * OC:(ot + 1) * OC],
                        start=(h == 0), stop=(h == HP - 1),
                    )
                o_sb = opool.tile([128, OC], F32, tag="osb", name="osb")
                hw = OC // 2
                nc.vector.tensor_scalar_mul(o_sb[:, 0:hw], o_ps[:, 0:hw], 1.0)
                nc.scalar.copy(out=o_sb[:, hw:OC], in_=o_ps[:, hw:OC])
                nc.sync.dma_start(
                    out=out_d[it * 128:(it + 1) * 128, ot * OC:(ot + 1) * OC],
                    in_=o_sb)

            for i, (h, si, g) in enumerate(glist):
                if i + LOOKAHEAD < len(glist):
                    issue_s_mask_exp(i + LOOKAHEAD)
                for _ in range(2):  # drain out-projection work into PE slack
                    if cwork:
                        c_group(*cwork.pop(0))
                issue_ctx(i)
                if g == si:  # last group of (h, si)
                    normalize(h, si)
                    if h == HP - 1:
                        cwork.extend(
                            (it, ot)
                            for it in range(2 * si, 2 * si + 2)
                            for ot in range(NO))
            while cwork:
                c_group(*cwork.pop(0))
        psS_cm.__exit__(None, None, None)

    return nc


def _prep_inputs(x, mask, cos, sin, wq, bq, wk, bk, wv, bv, wo, q_scale, k_scale):
    x2 = np.asarray(x, dtype=np.float32).reshape(T, EMB)
    # strip layout: row (it*128 + p), col (eb*128 + t) holds x[it*128+t, eb*128+p]
    xTt = x2.reshape(NT, 128, NE, 128).transpose(0, 3, 2, 1)
    xTt = np.ascontiguousarray(xTt).reshape(NT * 128, NE * 128).astype(BF)

    qs = np.asarray(q_scale, dtype=np.float32)
    ks = np.asarray(k_scale, dtype=np.float32)
    qs_rot = np.concatenate([qs[64:], qs[:64]])
    ks_rot = np.concatenate([ks[64:], ks[:64]])
    cos = np.asarray(cos, dtype=np.float32)
    sin = np.asarray(sin, dtype=np.float32)
    cosq = np.ascontiguousarray(cos * qs[None, :])
    sinq = np.ascontiguousarray(sin * qs_rot[None, :])
    cosk = np.ascontiguousarray(cos * ks[None, :])
    sink = np.ascontiguousarray(sin * ks_rot[None, :])

    jj = np.arange(128)[:, None]
    cc = np.arange(128)[None, :]
    tri = np.where(jj > cc, NEG, 0.0).astype(np.float32)
    z = np.zeros((128, 128), np.float32)
    mask256 = np.concatenate([tri, z, z, tri], axis=1)

    wq = np.asarray(wq, dtype=np.float32)
    wk = np.asarray(wk, dtype=np.float32)
    wv = np.asarray(wv, dtype=np.float32)
    wo = np.asarray(wo, dtype=np.float32)
    bq = np.asarray(bq, dtype=np.float32)
    bk = np.asarray(bk, dtype=np.float32)
    bv = np.asarray(bv, dtype=np.float32)

    in_maps = []
    for c in range(NCORES):
        wqkv_c = np.ascontiguousarray(
            np.concatenate([wq[:, c * QW:(c + 1) * QW],
                            wk[:, c * D:(c + 1) * D],
                            wv[:, c * D:(c + 1) * D]], axis=1)).astype(BF)
        wo_c = np.ascontiguousarray(wo[c * QW:(c + 1) * QW, :]).astype(BF)
        brow_c = np.concatenate(
            [bq[c * QW:(c + 1) * QW], bk[c * D:(c + 1) * D],
             bv[c * D:(c + 1) * D]]).reshape(1, WW).astype(BF)
        in_maps.append({
            "xT": xTt,
            "wqkv": wqkv_c,
            "wo": wo_c,
            "cosq": cosq, "sinq": sinq, "cosk": cosk, "sink": sink,
            "mask256": mask256,
            "ident": np.eye(128, dtype=BF),
            "brow": brow_c,
        })
    return in_maps


def _get_program(zero_bias=True):
    key = ("nc", zero_bias)
    if key not in _prog_cache:
        nc = _build_program(zero_bias)
        if not nc.is_finalized():
            nc.finalize()
        _prog_cache[key] = nc
    return _prog_cache[key]


def kernel(**inputs):
    zb = not (np.any(inputs["bq"]) or np.any(inputs["bk"])
              or np.any(inputs["bv"]))
    in_maps = _prep_inputs(**inputs)
    nc = _get_program(bool(zb))
    res = run_bass_kernel_spmd(nc, in_maps, list(range(NCORES)))
    out = np.zeros((T, EMB), dtype=np.float32)
    for r in res.results:
        out += np.asarray(r["out"], dtype=np.float32)
    return out.reshape(1, T, EMB)


# revision 7
# speedup vs baseline: 1.0025x; 1.0025x over previous
"""GQA attention kernel for Trainium2, tensor-parallel over heads on 8 cores.

Problem: B=1, T=2048, EMB=4096, H=32 query heads, G=8 KV groups, D=128.
Reference: q/k/v projections -> per-head RMS norm (q,k) -> RoPE (q,k) ->
causal GQA attention -> out projection.

Sharding: core c owns query heads [4c, 4c+4) and KV group c.  Each core
computes a partial output x @ W_o_rows for its heads; host sums the 8
partials (the all-reduce of the module's TP scheme).

Design notes:
  - softmax denominator is FREE: ctx matmuls use P-block [k,128q] as the
    stationary operand and [v | ones] (129 moving cols) as the moving one;
    den accumulates in psum col 128 of the same accumulation group.
  - normalization is a per-partition tensor_scalar multiply (q is the
    partition dim of the ctx psum tile) -- no partition broadcast needed.
  - attention runs in 256-query slices; causal sub-block skipping: the
    ctx matmul (qb, jb) is only issued when jb <= 2*si+qb, so the
    fully-masked P regions are never read; the mask is a [128,128]
    triangle added only on diagonal subblocks (post-matmul on DVE --
    PE accumulate does NOT see DVE-prewritten psum on real HW).
  - qkv biases fold into the projection accumulation as a K=1 matmul
    (ones[1,128] x brow[1,768]); when the biases are all zero (the
    common case) a specialized program variant skips them entirely.
  - each open psum accumulation group needs its own bank (zero region):
    8 banks = psS(2)+psS2(1) S-groups, ctxq0(1), ctxq1(1), psT2(1),
    psO(2); S-groups rotate 3-deep across psS/psS2.
  - phases: A (projections+rms+rope+transpose), then B (attention) and
    C (out projection) fused: C ot-groups are drained into the B stream
    (2 per group-iteration) so PE fills ACT-bound exp slack; the first
    B S/exp groups are issued during the last A tile.
  - the SP queue issues DMAs serially (blocking for the transfer), so
    DMA program order == arrival order: wqkv e0-3, x strip 0, e4-15,
    x strip 1, rope tables, consts, remaining wqkv; wo/mask issued at
    A-tile 2; exp table loaded once (Square/Copy live in every set).
    The last A tile copies q/kv psum to SBUF so the A psum banks (and
    with them the B pools) free before its rms/rope chain drains.
"""

import numpy as np
import ml_dtypes
from contextlib import ExitStack

import concourse.bacc as bacc
import concourse.mybir as mybir
from concourse.tile import TileContext
from concourse.bass_utils import run_bass_kernel_spmd

EMB, H, G, D, T = 4096, 32, 8, 128, 2048
EPS = 1e-6
NCORES = 8
HP = H // NCORES          # 4 query heads per core
NT = T // 128             # 16 t-tiles
NE = EMB // 128           # 32 e-tiles
QW = HP * D               # 512 = q width per core
KVW = 2 * D               # 256 = k|v width per core
WW = QW + KVW             # 768 = packed qkv width per core
SM_SCALE = 1.0 / float(np.sqrt(D))
NEG = -1e9
OC = 512                  # phase C output column tile width
NO = EMB // OC            # 16 output column tiles

F32 = mybir.dt.float32
BF16 = mybir.dt.bfloat16
BF = ml_dtypes.bfloat16

_prog_cache = {}


def _build_program(zero_bias):
    nc = bacc.Bacc()

    xT_d = nc.declare_dram_parameter("xT", [NT * 128, NE * 128], BF16, isOutput=False)
    wqkv_d = nc.declare_dram_parameter("wqkv", [NE * 128, WW], BF16, isOutput=False)
    wo_d = nc.declare_dram_parameter("wo", [HP * 128, EMB], BF16, isOutput=False)
    cq_d = nc.declare_dram_parameter("cosq", [NT * 128, 128], F32, isOutput=False)
    sq_d = nc.declare_dram_parameter("sinq", [NT * 128, 128], F32, isOutput=False)
    ck_d = nc.declare_dram_parameter("cosk", [NT * 128, 128], F32, isOutput=False)
    sk_d = nc.declare_dram_parameter("sink", [NT * 128, 128], F32, isOutput=False)
    mask_d = nc.declare_dram_parameter("mask256", [128, 512], F32, isOutput=False)
    ident_d = nc.declare_dram_parameter("ident", [128, 128], BF16, isOutput=False)
    brow_d = (None if zero_bias else
              nc.declare_dram_parameter("brow", [1, WW], BF16, isOutput=False))
    out_d = nc.declare_dram_parameter("out", [T, EMB], F32, isOutput=True)

    with TileContext(nc) as tc, ExitStack() as ctx:
        consts = ctx.enter_context(tc.tile_pool(name="consts", bufs=1))
        wpool = ctx.enter_context(tc.tile_pool(name="wpool", bufs=1))
        xpool = ctx.enter_context(tc.tile_pool(name="xpool", bufs=2))
        cspool = ctx.enter_context(tc.tile_pool(name="cspool", bufs=2))
        scratch = ctx.enter_context(tc.tile_pool(name="scratch", bufs=8))
        small = ctx.enter_context(tc.tile_pool(name="small", bufs=12))
        resid = ctx.enter_context(tc.tile_pool(name="resid", bufs=1))
        ppool = ctx.enter_context(tc.tile_pool(name="ppool", bufs=6))
        opool = ctx.enter_context(tc.tile_pool(name="opool", bufs=6))

        # ---- early DMAs: the SP queue issues serially, so order = arrival.
        # First-needed first: wqkv e=0-3, x strip 0, more wqkv, x strip 1,
        # rope tables for tiles 0-1, brow/ident, remaining wqkv strips.
        TILE_ORDER = list(range(NT))
        wqkv_sb = [wpool.tile([128, WW], BF16, tag=f"wqkv{e}", name=f"wqkv{e}")
                   for e in range(NE)]

        def _wq_dma(e):
            nc.sync.dma_start(out=wqkv_sb[e],
                              in_=wqkv_d[e * 128:(e + 1) * 128, :])

        for e in range(4):
            _wq_dma(e)
        xs_hoist = {}
        xs = xpool.tile([128, NE * 128], BF16, tag="xstrip", name="xstrip")
        nc.sync.dma_start(out=xs, in_=xT_d[0:128, :])
        xs_hoist[0] = xs
        for e in range(4, 11):
            _wq_dma(e)
        xs = xpool.tile([128, NE * 128], BF16, tag="xstrip", name="xstrip")
        nc.sync.dma_start(out=xs, in_=xT_d[128:256, :])
        xs_hoist[1] = xs
        cs_hoist = {}
        for it in TILE_ORDER[:2]:
            tiles = []
            for nm, d in (("cq", cq_d), ("sq", sq_d), ("ck", ck_d), ("sk", sk_d)):
                t = cspool.tile([128, 128], F32, tag=nm, name=nm)
                nc.sync.dma_start(out=t, in_=d[it * 128:(it + 1) * 128, :])
                tiles.append(t)
            cs_hoist[it] = tiles
        ident = consts.tile([128, 128], BF16, tag="ident", name="ident")
        nc.sync.dma_start(out=ident, in_=ident_d[:, :])
        eps_t = consts.tile([128, 1], F32, tag="eps", name="eps")
        nc.vector.memset(eps_t, EPS)
        # pre-warm the Sqrt act-func set during the DMA dead time so the
        # 1.3us table loads stay off the first tile's rms chain (Square and
        # Copy live in the same set; only Exp needs another load, at B start)
        warm_t = consts.tile([128, 1], F32, tag="warm", name="warm")
        nc.scalar.activation(out=warm_t, in_=eps_t,
                             func=mybir.ActivationFunctionType.Sqrt)
        if not zero_bias:
            brow = consts.tile([1, WW], BF16, tag="brow", name="brow")
            nc.sync.dma_start(out=brow, in_=brow_d[:, :])
            ones1 = consts.tile([1, 128], BF16, tag="ones1", name="ones1")
            nc.vector.memset(ones1, 1.0)
        for e in range(11, NE):
            _wq_dma(e)
        mask_sb = consts.tile([128, 512], F32, tag="mask", name="mask")
        wo_sb = [wpool.tile([128, EMB], BF16, tag=f"wo{h}", name=f"wo{h}")
                 for h in range(HP)]

        # resident activations
        qT = [resid.tile([128, T], BF16, tag=f"qT{h}", name=f"qT{h}") for h in range(HP)]
        kT = resid.tile([128, T], BF16, tag="kT", name="kT")
        vone = [resid.tile([128, 129], BF16, tag=f"v{j}", name=f"v{j}") for j in range(NT)]
        ctxT = [resid.tile([128, T], BF16, tag=f"ctxT{h}", name=f"ctxT{h}") for h in range(HP)]
        for j in range(NT):
            nc.vector.memset(vone[j][:, 128:129], 1.0)

        # ---------------- Phases ------------------------------------------
        # A: projections + rms + rope + transpose.  B: attention (256-query
        # slices, P-stationary ctx matmuls with a ones column for the free
        # softmax denominator).  C: out projection, interleaved into B's
        # stream so PE fills ACT-bound slack.  The first B groups are issued
        # during late phase A (psS opens alongside the A pools).
        psS_cm = tc.tile_pool(name="psS", bufs=2, space="PSUM")
        psA_cm = tc.tile_pool(name="psA", bufs=2, space="PSUM")
        psKV_cm = tc.tile_pool(name="psKV", bufs=2, space="PSUM")
        psT_cm = tc.tile_pool(name="psT", bufs=2, space="PSUM")
        psS = psS_cm.__enter__()
        psA = psA_cm.__enter__()
        psKV = psKV_cm.__enter__()
        psT = psT_cm.__enter__()

        NSI = 8
        glist = []
        for si in range(NSI):
            for h in range(HP):
                glist.extend((h, si, g) for g in range(si + 1))
        pend_p = {}
        ctx_tiles = {}
        psS2 = [None]

        def issue_s_mask_exp(idx):
            h, si, g = glist[idx]
            pool = psS if idx % 3 < 2 else psS2[0]
            s_g = pool.tile([128, 512], F32, tag="sg", name="sg")
            for half in range(2):
                jb = 2 * g + half
                nc.tensor.matmul(
                    s_g[:, half * 256:(half + 1) * 256],
                    kT[:, jb * 128:(jb + 1) * 128],
                    qT[h][:, si * 256:(si + 1) * 256],
                    start=True, stop=True,
                )
            for half in range(2):
                kk = 2 * g + half - 2 * si
                if kk >= 0:  # diagonal: triangular mask on its subblock
                    off = half * 256 + kk * 128
                    nc.vector.tensor_add(
                        s_g[:, off:off + 128], s_g[:, off:off + 128],
                        mask_sb[:, kk * 256 + kk * 128:kk * 256 + (kk + 1) * 128])
            p_g = ppool.tile([128, 512], BF16, tag="pt", name="pt")
            nc.scalar.activation(
                out=p_g, in_=s_g,
                func=mybir.ActivationFunctionType.Exp,
                scale=SM_SCALE,
            )
            pend_p[idx] = p_g

        LOOKAHEAD = 3

        def phase_a():
            for idx, it in enumerate(TILE_ORDER):
                q_ps = psA.tile([128, QW], F32, tag="aq", name="aq")
                kv_ps = psKV.tile([128, KVW], F32, tag="akv", name="akv")
                if it in xs_hoist:
                    xstrip = xs_hoist.pop(it)
                else:
                    xstrip = xpool.tile([128, NE * 128], BF16, tag="xstrip",
                                        name="xstrip")
                    nc.sync.dma_start(out=xstrip, in_=xT_d[it * 128:(it + 1) * 128, :])
                for e in range(NE):
                    xt = xstrip[:, e * 128:(e + 1) * 128]
                    last = zero_bias and e == NE - 1
                    nc.tensor.matmul(kv_ps, xt, wqkv_sb[e][:, QW:WW],
                                     start=(e == 0), stop=last)
                for e in range(NE):
                    xt = xstrip[:, e * 128:(e + 1) * 128]
                    last = zero_bias and e == NE - 1
                    nc.tensor.matmul(q_ps, xt, wqkv_sb[e][:, 0:QW],
                                     start=(e == 0), stop=last)
                if not zero_bias:
                    # bias add fused into the accumulation as a K=1 matmul
                    nc.tensor.matmul(q_ps, ones1, brow[:, 0:QW],
                                     start=False, stop=True)
                    nc.tensor.matmul(kv_ps, ones1, brow[:, QW:WW],
                                     start=False, stop=True)
                if idx == NT - 1:
                    # pre-fill the attention pipeline; si=0 needs only
                    # qT/kT t-tiles 0-1, ready long ago
                    issue_s_mask_exp(0)
                    issue_s_mask_exp(1)

                if it in cs_hoist:
                    cq, sq, ck, sk = cs_hoist.pop(it)
                else:
                    cq = cspool.tile([128, 128], F32, tag="cq", name="cq")
                    nc.sync.dma_start(out=cq, in_=cq_d[it * 128:(it + 1) * 128, :])
                    sq = cspool.tile([128, 128], F32, tag="sq", name="sq")
                    nc.sync.dma_start(out=sq, in_=sq_d[it * 128:(it + 1) * 128, :])
                    ck = cspool.tile([128, 128], F32, tag="ck", name="ck")
                    nc.sync.dma_start(out=ck, in_=ck_d[it * 128:(it + 1) * 128, :])
                    sk = cspool.tile([128, 128], F32, tag="sk", name="sk")
                    nc.sync.dma_start(out=sk, in_=sk_d[it * 128:(it + 1) * 128, :])
                if idx == 2:
                    nc.sync.dma_start(out=mask_sb, in_=mask_d[:, :])
                    for hh in range(HP):
                        nc.sync.dma_start(
                            out=wo_sb[hh], in_=wo_d[hh * 128:(hh + 1) * 128, :])

                if idx == NT - 1:
                    # free the A psum banks early: the B pools allocate them
                    qsb = scratch.tile([128, QW], F32, tag="qsb", name="qsb")
                    nc.scalar.copy(out=qsb, in_=q_ps)
                    kvsb = scratch.tile([128, KVW], F32, tag="kvsb", name="kvsb")
                    nc.scalar.copy(out=kvsb, in_=kv_ps)
                    q_src, kv_src = qsb, kvsb
                else:
                    q_src, kv_src = q_ps, kv_ps
                nc.vector.tensor_scalar_mul(
                    vone[it][:, 0:128], kv_src[:, 128:256], 1.0)
                for b in (HP, 0, 1, 2, 3):  # k first, then q heads
                    if b < HP:
                        src = q_src[:, b * 128:(b + 1) * 128]
                        c_t, s_t = cq[:, :], sq[:, :]
                    else:
                        src = kv_src[:, 0:128]
                        c_t, s_t = ck[:, :], sk[:, :]
                    sqout = scratch.tile([128, 128], F32, tag="sqout", name="sqout")
                    sqacc = small.tile([128, 1], F32, tag="sqacc", name="sqacc")
                    nc.scalar.activation(
                        out=sqout, in_=src,
                        func=mybir.ActivationFunctionType.Square,
                        accum_out=sqacc,
                    )
                    rstd = small.tile([128, 1], F32, tag="rstd", name="rstd")
                    nc.scalar.activation(
                        out=rstd, in_=sqacc,
                        func=mybir.ActivationFunctionType.Sqrt,
                        bias=eps_t, scale=1.0 / D,
                    )
                    nc.vector.reciprocal(rstd, rstd)
                    # rope: out1 = x1*c1 - x2*s1 ; out2 = x2*c2 + x1*s2
                    rt = scratch.tile([128, 128], F32, tag="rt", name="rt")
                    m1 = scratch.tile([128, 64], F32, tag="m1", name="m1")
                    nc.vector.tensor_mul(rt[:, 0:64], src[:, 0:64], c_t[:, 0:64])
                    nc.vector.tensor_mul(m1, src[:, 64:128], s_t[:, 0:64])
                    nc.vector.tensor_sub(rt[:, 0:64], rt[:, 0:64], m1)
                    m2 = scratch.tile([128, 64], F32, tag="m2", name="m2")
                    nc.vector.tensor_mul(rt[:, 64:128], src[:, 64:128], c_t[:, 64:128])
                    nc.vector.tensor_mul(m2, src[:, 0:64], s_t[:, 64:128])
                    nc.vector.tensor_add(rt[:, 64:128], rt[:, 64:128], m2)
                    rb = scratch.tile([128, 128], BF16, tag="rb", name="rb")
                    nc.vector.tensor_scalar_mul(rb, rt, rstd)
                    tp = psT.tile([128, 128], BF16, tag="tp", name="tp")
                    nc.tensor.transpose(tp, rb, ident)
                    dst = qT[b] if b < HP else kT
                    nc.scalar.copy(out=dst[:, it * 128:(it + 1) * 128], in_=tp)

        phase_a()
        psT_cm.__exit__(None, None, None)
        psKV_cm.__exit__(None, None, None)
        psA_cm.__exit__(None, None, None)

        # ---------------- Phases B+C fused, si-major ------------------------
        # Each open psum accumulation group needs its own bank (zero region):
        # ctx q-blocks live in separate single-buffered pools.
        with tc.tile_pool(name="psS2", bufs=1, space="PSUM") as psS2_pool, \
             tc.tile_pool(name="psC0", bufs=1, space="PSUM") as psC0, \
             tc.tile_pool(name="psC1", bufs=1, space="PSUM") as psC1, \
             tc.tile_pool(name="psT2", bufs=1, space="PSUM") as psT2, \
             tc.tile_pool(name="psO", bufs=2, space="PSUM") as psO:

            cwork = []
            psS2[0] = psS2_pool
            issue_s_mask_exp(2)

            def issue_ctx(idx):
                h, si, g = glist[idx]
                p_g = pend_p.pop(idx)
                if g == 0:
                    c0 = psC0.tile([128, 129], F32, tag="ctxq0", name="ctxq0")
                    c1 = psC1.tile([128, 129], F32, tag="ctxq1", name="ctxq1")
                    ctx_tiles[(h, si)] = (c0, c1)
                cts = ctx_tiles[(h, si)]
                for half in range(2):
                    jb = 2 * g + half
                    kk = jb - 2 * si
                    for qb in range(2):
                        if kk > qb:
                            continue  # fully-masked subblock: skip
                        nc.tensor.matmul(
                            cts[qb],
                            p_g[:, half * 256 + qb * 128:half * 256 + (qb + 1) * 128],
                            vone[jb],
                            start=(jb == 0), stop=(jb == 2 * si + qb),
                        )

            def normalize(h, si):
                cts = ctx_tiles.pop((h, si))
                for qb in range(2):
                    ct = cts[qb]
                    rden = small.tile([128, 1], F32, tag="rden", name="rden")
                    nc.vector.reciprocal(rden, ct[:, 128:129])
                    cbs = scratch.tile([128, 128], BF16, tag="cbs", name="cbs")
                    nc.vector.tensor_scalar_mul(cbs, ct[:, 0:128], rden)
                    tp = psT2.tile([128, 128], BF16, tag="tp2", name="tp2")
                    nc.tensor.transpose(tp, cbs, ident)
                    nc.vector.tensor_scalar_mul(
                        ctxT[h][:, si * 256 + qb * 128:si * 256 + (qb + 1) * 128],
                        tp, 1.0)

            def c_group(it, ot):
                o_ps = psO.tile([128, OC], F32, tag="ops", name="ops")
                for h in range(HP):
                    nc.tensor.matmul(
                        o_ps,
                        ctxT[h][:, it * 128:(it + 1) * 128],
                        wo_sb[h][:, ot * OC:(ot + 1) * OC],
                        start=(h == 0), stop=(h == HP - 1),
                    )
                o_sb = opool.tile([128, OC], F32, tag="osb", name="osb")
                hw = OC // 2
                nc.vector.tensor_scalar_mul(o_sb[:, 0:hw], o_ps[:, 0:hw], 1.0)
                nc.scalar.copy(out=o_sb[:, hw:OC], in_=o_ps[:, hw:OC])
                nc.sync.dma_start(
                    out=out_d[it * 128:(it + 1) * 128, ot * OC:(ot + 1) * OC],
                    in_=o_sb)

            for i, (h, si, g) in enumerate(glist):
                if i + LOOKAHEAD < len(glist):
                    issue_s_mask_exp(i + LOOKAHEAD)
                for _ in range(3):  # drain out-projection work into PE slack
                    if cwork:
                        c_group(*cwork.pop(0))
                issue_ctx(i)
                if g == si:  # last group of (h, si)
                    normalize(h, si)
                    if h == HP - 1:
                        cwork.extend(
                            (it, ot)
                            for it in range(2 * si, 2 * si + 2)
                            for ot in range(NO))
            while cwork:
                c_group(*cwork.pop(0))
        psS_cm.__exit__(None, None, None)

    return nc


def _prep_inputs(x, mask, cos, sin, wq, bq, wk, bk, wv, bv, wo, q_scale, k_scale):
    x2 = np.asarray(x, dtype=np.float32).reshape(T, EMB)
    # strip layout: row (it*128 + p), col (eb*128 + t) holds x[it*128+t, eb*128+p]
    xTt = x2.reshape(NT, 128, NE, 128).transpose(0, 3, 2, 1)
    xTt = np.ascontiguousarray(xTt).reshape(NT * 128, NE * 128).astype(BF)

    qs = np.asarray(q_scale, dtype=np.float32)
    ks = np.asarray(k_scale, dtype=np.float32)
    qs_rot = np.concatenate([qs[64:], qs[:64]])
    ks_rot = np.concatenate([ks[64:], ks[:64]])
    cos = np.asarray(cos, dtype=np.float32)
    sin = np.asarray(sin, dtype=np.float32)
    cosq = np.ascontiguousarray(cos * qs[None, :])
    sinq = np.ascontiguousarray(sin * qs_rot[None, :])
    cosk = np.ascontiguousarray(cos * ks[None, :])
    sink = np.ascontiguousarray(sin * ks_rot[None, :])

    jj = np.arange(128)[:, None]
    cc = np.arange(128)[None, :]
    tri = np.where(jj > cc, NEG, 0.0).astype(np.float32)
    z = np.zeros((128, 128), np.float32)
    mask256 = np.concatenate([tri, z, z, tri], axis=1)

    wq = np.asarray(wq, dtype=np.float32)
    wk = np.asarray(wk, dtype=np.float32)
    wv = np.asarray(wv, dtype=np.float32)
    wo = np.asarray(wo, dtype=np.float32)
    bq = np.asarray(bq, dtype=np.float32)
    bk = np.asarray(bk, dtype=np.float32)
    bv = np.asarray(bv, dtype=np.float32)

    in_maps = []
    for c in range(NCORES):
        wqkv_c = np.ascontiguousarray(
            np.concatenate([wq[:, c * QW:(c + 1) * QW],
                            wk[:, c * D:(c + 1) * D],
                            wv[:, c * D:(c + 1) * D]], axis=1)).astype(BF)
        wo_c = np.ascontiguousarray(wo[c * QW:(c + 1) * QW, :]).astype(BF)
        brow_c = np.concatenate(
            [bq[c * QW:(c + 1) * QW], bk[c * D:(c + 1) * D],
             bv[c * D:(c + 1) * D]]).reshape(1, WW).astype(BF)
        in_maps.append({
            "xT": xTt,
            "wqkv": wqkv_c,
            "wo": wo_c,
            "cosq": cosq, "sinq": sinq, "cosk": cosk, "sink": sink,
            "mask256": mask256,
            "ident": np.eye(128, dtype=BF),
            "brow": brow_c,
        })
    return in_maps


def _get_program(zero_bias=True):
    key = ("nc", zero_bias)
    if key not in _prog_cache:
        nc = _build_program(zero_bias)
        if not nc.is_finalized():
            nc.finalize()
        _prog_cache[key] = nc
    return _prog_cache[key]


def kernel(**inputs):
    zb = not (np.any(inputs["bq"]) or np.any(inputs["bk"])
              or np.any(inputs["bv"]))
    in_maps = _prep_inputs(**inputs)
    nc = _get_program(bool(zb))
    res = run_bass_kernel_spmd(nc, in_maps, list(range(NCORES)))
    out = np.zeros((T, EMB), dtype=np.float32)
    for r in res.results:
        out += np.asarray(r["out"], dtype=np.float32)
    return out.reshape(1, T, EMB)


# revision 8
# speedup vs baseline: 1.0041x; 1.0016x over previous
"""GQA attention kernel for Trainium2, tensor-parallel over heads on 8 cores.

Problem: B=1, T=2048, EMB=4096, H=32 query heads, G=8 KV groups, D=128.
Reference: q/k/v projections -> per-head RMS norm (q,k) -> RoPE (q,k) ->
causal GQA attention -> out projection.

Sharding: core c owns query heads [4c, 4c+4) and KV group c.  Each core
computes a partial output x @ W_o_rows for its heads; host sums the 8
partials (the all-reduce of the module's TP scheme).

Design notes:
  - softmax denominator is FREE: ctx matmuls use P-block [k,128q] as the
    stationary operand and [v | ones] (129 moving cols) as the moving one;
    den accumulates in psum col 128 of the same accumulation group.
  - normalization is a per-partition tensor_scalar multiply (q is the
    partition dim of the ctx psum tile) -- no partition broadcast needed.
  - attention runs in 256-query slices; causal sub-block skipping: the
    ctx matmul (qb, jb) is only issued when jb <= 2*si+qb, so the
    fully-masked P regions are never read; the mask is a [128,128]
    triangle added only on diagonal subblocks (post-matmul on DVE --
    PE accumulate does NOT see DVE-prewritten psum on real HW).
  - qkv biases fold into the projection accumulation as a K=1 matmul
    (ones[1,128] x brow[1,768]); when the biases are all zero (the
    common case) a specialized program variant skips them entirely.
  - each open psum accumulation group needs its own bank (zero region):
    8 banks = psS(2)+psS2(1) S-groups, ctxq0(1), ctxq1(1), psT2(1),
    psO(2); S-groups rotate 3-deep across psS/psS2.
  - phases: A (projections+rms+rope+transpose), then B (attention) and
    C (out projection) fused: C ot-groups are drained into the B stream
    (2 per group-iteration) so PE fills ACT-bound exp slack; the first
    B S/exp groups are issued during the last A tile.
  - the SP queue issues DMAs serially (blocking for the transfer), so
    DMA program order == arrival order: wqkv e0-3, x strip 0, e4-15,
    x strip 1, rope tables, consts, remaining wqkv; wo/mask issued at
    A-tile 2; exp table loaded once (Square/Copy live in every set).
    The last A tile copies q/kv psum to SBUF so the A psum banks (and
    with them the B pools) free before its rms/rope chain drains.
"""

import numpy as np
import ml_dtypes
from contextlib import ExitStack

import concourse.bacc as bacc
import concourse.mybir as mybir
from concourse.tile import TileContext
from concourse.bass_utils import run_bass_kernel_spmd

EMB, H, G, D, T = 4096, 32, 8, 128, 2048
EPS = 1e-6
NCORES = 8
HP = H // NCORES          # 4 query heads per core
NT = T // 128             # 16 t-tiles
NE = EMB // 128           # 32 e-tiles
QW = HP * D               # 512 = q width per core
KVW = 2 * D               # 256 = k|v width per core
WW = QW + KVW             # 768 = packed qkv width per core
SM_SCALE = 1.0 / float(np.sqrt(D))
NEG = -1e9
OC = 512                  # phase C output column tile width
NO = EMB // OC            # 16 output column tiles

F32 = mybir.dt.float32
BF16 = mybir.dt.bfloat16
BF = ml_dtypes.bfloat16

_prog_cache = {}


def _build_program(zero_bias):
    nc = bacc.Bacc()

    xT_d = nc.declare_dram_parameter("xT", [NT * 128, NE * 128], BF16, isOutput=False)
    wqkv_d = nc.declare_dram_parameter("wqkv", [NE * 128, WW], BF16, isOutput=False)
    wo_d = nc.declare_dram_parameter("wo", [HP * 128, EMB], BF16, isOutput=False)
    cq_d = nc.declare_dram_parameter("cosq", [NT * 128, 128], F32, isOutput=False)
    sq_d = nc.declare_dram_parameter("sinq", [NT * 128, 128], F32, isOutput=False)
    ck_d = nc.declare_dram_parameter("cosk", [NT * 128, 128], F32, isOutput=False)
    sk_d = nc.declare_dram_parameter("sink", [NT * 128, 128], F32, isOutput=False)
    mask_d = nc.declare_dram_parameter("mask256", [128, 512], F32, isOutput=False)
    ident_d = nc.declare_dram_parameter("ident", [128, 128], BF16, isOutput=False)
    brow_d = (None if zero_bias else
              nc.declare_dram_parameter("brow", [1, WW], BF16, isOutput=False))
    out_d = nc.declare_dram_parameter("out", [T, EMB], F32, isOutput=True)

    with TileContext(nc) as tc, ExitStack() as ctx:
        consts = ctx.enter_context(tc.tile_pool(name="consts", bufs=1))
        wpool = ctx.enter_context(tc.tile_pool(name="wpool", bufs=1))
        xpool = ctx.enter_context(tc.tile_pool(name="xpool", bufs=2))
        cspool = ctx.enter_context(tc.tile_pool(name="cspool", bufs=2))
        scratch = ctx.enter_context(tc.tile_pool(name="scratch", bufs=8))
        small = ctx.enter_context(tc.tile_pool(name="small", bufs=12))
        resid = ctx.enter_context(tc.tile_pool(name="resid", bufs=1))
        ppool = ctx.enter_context(tc.tile_pool(name="ppool", bufs=6))
        opool = ctx.enter_context(tc.tile_pool(name="opool", bufs=6))

        # ---- early DMAs: the SP queue issues serially, so order = arrival.
        # First-needed first: wqkv e=0-3, x strip 0, more wqkv, x strip 1,
        # rope tables for tiles 0-1, brow/ident, remaining wqkv strips.
        TILE_ORDER = list(range(NT))
        wqkv_sb = [wpool.tile([128, WW], BF16, tag=f"wqkv{e}", name=f"wqkv{e}")
                   for e in range(NE)]

        def _wq_dma(e):
            nc.sync.dma_start(out=wqkv_sb[e],
                              in_=wqkv_d[e * 128:(e + 1) * 128, :])

        for e in range(4):
            _wq_dma(e)
        xs_hoist = {}
        xs = xpool.tile([128, NE * 128], BF16, tag="xstrip", name="xstrip")
        nc.sync.dma_start(out=xs, in_=xT_d[0:128, :])
        xs_hoist[0] = xs
        for e in range(4, 11):
            _wq_dma(e)
        xs = xpool.tile([128, NE * 128], BF16, tag="xstrip", name="xstrip")
        nc.sync.dma_start(out=xs, in_=xT_d[128:256, :])
        xs_hoist[1] = xs
        cs_hoist = {}
        for it in TILE_ORDER[:2]:
            tiles = []
            for nm, d in (("cq", cq_d), ("sq", sq_d), ("ck", ck_d), ("sk", sk_d)):
                t = cspool.tile([128, 128], F32, tag=nm, name=nm)
                nc.sync.dma_start(out=t, in_=d[it * 128:(it + 1) * 128, :])
                tiles.append(t)
            cs_hoist[it] = tiles
        ident = consts.tile([128, 128], BF16, tag="ident", name="ident")
        nc.sync.dma_start(out=ident, in_=ident_d[:, :])
        eps_t = consts.tile([128, 1], F32, tag="eps", name="eps")
        nc.vector.memset(eps_t, EPS)
        # pre-warm the Sqrt act-func set during the DMA dead time so the
        # 1.3us table loads stay off the first tile's rms chain (Square and
        # Copy live in the same set; only Exp needs another load, at B start)
        warm_t = consts.tile([128, 1], F32, tag="warm", name="warm")
        nc.scalar.activation(out=warm_t, in_=eps_t,
                             func=mybir.ActivationFunctionType.Sqrt)
        # PE p-state warm-up: keep the PE continuously busy on dummy matmuls
        # through the initial DMA wait so the clock is fully ramped when the
        # first projection matmul's inputs land. A late ramp costs ~2.5us;
        # if the DMAs beat the dummies the loss is bounded by one dummy.
        dmA = consts.tile([128, 128], BF16, tag="dmA", name="dmA")
        nc.vector.memset(dmA, 0.0)
        dmB = consts.tile([128, 512], BF16, tag="dmB", name="dmB")
        nc.vector.memset(dmB, 0.0)
        if not zero_bias:
            brow = consts.tile([1, WW], BF16, tag="brow", name="brow")
            nc.sync.dma_start(out=brow, in_=brow_d[:, :])
            ones1 = consts.tile([1, 128], BF16, tag="ones1", name="ones1")
            nc.vector.memset(ones1, 1.0)
        for e in range(11, NE):
            _wq_dma(e)
        mask_sb = consts.tile([128, 512], F32, tag="mask", name="mask")
        wo_sb = [wpool.tile([128, EMB], BF16, tag=f"wo{h}", name=f"wo{h}")
                 for h in range(HP)]

        # resident activations
        qT = [resid.tile([128, T], BF16, tag=f"qT{h}", name=f"qT{h}") for h in range(HP)]
        kT = resid.tile([128, T], BF16, tag="kT", name="kT")
        vone = [resid.tile([128, 129], BF16, tag=f"v{j}", name=f"v{j}") for j in range(NT)]
        ctxT = [resid.tile([128, T], BF16, tag=f"ctxT{h}", name=f"ctxT{h}") for h in range(HP)]
        for j in range(NT):
            nc.vector.memset(vone[j][:, 128:129], 1.0)

        # ---------------- Phases ------------------------------------------
        # A: projections + rms + rope + transpose.  B: attention (256-query
        # slices, P-stationary ctx matmuls with a ones column for the free
        # softmax denominator).  C: out projection, interleaved into B's
        # stream so PE fills ACT-bound slack.  The first B groups are issued
        # during late phase A (psS opens alongside the A pools).
        psS_cm = tc.tile_pool(name="psS", bufs=2, space="PSUM")
        psA_cm = tc.tile_pool(name="psA", bufs=2, space="PSUM")
        psKV_cm = tc.tile_pool(name="psKV", bufs=2, space="PSUM")
        psT_cm = tc.tile_pool(name="psT", bufs=2, space="PSUM")
        psS = psS_cm.__enter__()
        psA = psA_cm.__enter__()
        psKV = psKV_cm.__enter__()
        psT = psT_cm.__enter__()

        NSI = 8
        glist = []
        for si in range(NSI):
            for h in range(HP):
                glist.extend((h, si, g) for g in range(si + 1))
        pend_p = {}
        ctx_tiles = {}
        psS2 = [None]

        def issue_s_mask_exp(idx):
            h, si, g = glist[idx]
            pool = psS if idx % 3 < 2 else psS2[0]
            s_g = pool.tile([128, 512], F32, tag="sg", name="sg")
            for half in range(2):
                jb = 2 * g + half
                nc.tensor.matmul(
                    s_g[:, half * 256:(half + 1) * 256],
                    kT[:, jb * 128:(jb + 1) * 128],
                    qT[h][:, si * 256:(si + 1) * 256],
                    start=True, stop=True,
                )
            for half in range(2):
                kk = 2 * g + half - 2 * si
                if kk >= 0:  # diagonal: triangular mask on its subblock
                    off = half * 256 + kk * 128
                    nc.vector.tensor_add(
                        s_g[:, off:off + 128], s_g[:, off:off + 128],
                        mask_sb[:, kk * 256 + kk * 128:kk * 256 + (kk + 1) * 128])
            p_g = ppool.tile([128, 512], BF16, tag="pt", name="pt")
            nc.scalar.activation(
                out=p_g, in_=s_g,
                func=mybir.ActivationFunctionType.Exp,
                scale=SM_SCALE,
            )
            pend_p[idx] = p_g

        LOOKAHEAD = 3

        def phase_a():
            for w in range(26):
                wps = psS.tile([128, 512], F32, tag="sg", name="sg")
                nc.tensor.matmul(wps, dmA, dmB, start=True, stop=True)
            for idx, it in enumerate(TILE_ORDER):
                q_ps = psA.tile([128, QW], F32, tag="aq", name="aq")
                kv_ps = psKV.tile([128, KVW], F32, tag="akv", name="akv")
                if it in xs_hoist:
                    xstrip = xs_hoist.pop(it)
                else:
                    xstrip = xpool.tile([128, NE * 128], BF16, tag="xstrip",
                                        name="xstrip")
                    nc.sync.dma_start(out=xstrip, in_=xT_d[it * 128:(it + 1) * 128, :])
                for e in range(NE):
                    xt = xstrip[:, e * 128:(e + 1) * 128]
                    last = zero_bias and e == NE - 1
                    nc.tensor.matmul(kv_ps, xt, wqkv_sb[e][:, QW:WW],
                                     start=(e == 0), stop=last)
                for e in range(NE):
                    xt = xstrip[:, e * 128:(e + 1) * 128]
                    last = zero_bias and e == NE - 1
                    nc.tensor.matmul(q_ps, xt, wqkv_sb[e][:, 0:QW],
                                     start=(e == 0), stop=last)
                if not zero_bias:
                    # bias add fused into the accumulation as a K=1 matmul
                    nc.tensor.matmul(q_ps, ones1, brow[:, 0:QW],
                                     start=False, stop=True)
                    nc.tensor.matmul(kv_ps, ones1, brow[:, QW:WW],
                                     start=False, stop=True)
                if idx == NT - 1:
                    # pre-fill the attention pipeline; si=0 needs only
                    # qT/kT t-tiles 0-1, ready long ago
                    issue_s_mask_exp(0)
                    issue_s_mask_exp(1)

                if it in cs_hoist:
                    cq, sq, ck, sk = cs_hoist.pop(it)
                else:
                    cq = cspool.tile([128, 128], F32, tag="cq", name="cq")
                    nc.sync.dma_start(out=cq, in_=cq_d[it * 128:(it + 1) * 128, :])
                    sq = cspool.tile([128, 128], F32, tag="sq", name="sq")
                    nc.sync.dma_start(out=sq, in_=sq_d[it * 128:(it + 1) * 128, :])
                    ck = cspool.tile([128, 128], F32, tag="ck", name="ck")
                    nc.sync.dma_start(out=ck, in_=ck_d[it * 128:(it + 1) * 128, :])
                    sk = cspool.tile([128, 128], F32, tag="sk", name="sk")
                    nc.sync.dma_start(out=sk, in_=sk_d[it * 128:(it + 1) * 128, :])
                if idx == 2:
                    nc.sync.dma_start(out=mask_sb, in_=mask_d[:, :])
                    for hh in range(HP):
                        nc.sync.dma_start(
                            out=wo_sb[hh], in_=wo_d[hh * 128:(hh + 1) * 128, :])

                if idx == NT - 1:
                    # free the A psum banks early: the B pools allocate them
                    qsb = scratch.tile([128, QW], F32, tag="qsb", name="qsb")
                    nc.scalar.copy(out=qsb, in_=q_ps)
                    kvsb = scratch.tile([128, KVW], F32, tag="kvsb", name="kvsb")
                    nc.scalar.copy(out=kvsb, in_=kv_ps)
                    q_src, kv_src = qsb, kvsb
                else:
                    q_src, kv_src = q_ps, kv_ps
                nc.vector.tensor_scalar_mul(
                    vone[it][:, 0:128], kv_src[:, 128:256], 1.0)
                for b in (HP, 0, 1, 2, 3):  # k first, then q heads
                    if b < HP:
                        src = q_src[:, b * 128:(b + 1) * 128]
                        c_t, s_t = cq[:, :], sq[:, :]
                    else:
                        src = kv_src[:, 0:128]
                        c_t, s_t = ck[:, :], sk[:, :]
                    sqout = scratch.tile([128, 128], F32, tag="sqout", name="sqout")
                    sqacc = small.tile([128, 1], F32, tag="sqacc", name="sqacc")
                    nc.scalar.activation(
                        out=sqout, in_=src,
                        func=mybir.ActivationFunctionType.Square,
                        accum_out=sqacc,
                    )
                    rstd = small.tile([128, 1], F32, tag="rstd", name="rstd")
                    nc.scalar.activation(
                        out=rstd, in_=sqacc,
                        func=mybir.ActivationFunctionType.Sqrt,
                        bias=eps_t, scale=1.0 / D,
                    )
                    nc.vector.reciprocal(rstd, rstd)
                    # rope: out1 = x1*c1 - x2*s1 ; out2 = x2*c2 + x1*s2
                    rt = scratch.tile([128, 128], F32, tag="rt", name="rt")
                    m1 = scratch.tile([128, 64], F32, tag="m1", name="m1")
                    nc.vector.tensor_mul(rt[:, 0:64], src[:, 0:64], c_t[:, 0:64])
                    nc.vector.tensor_mul(m1, src[:, 64:128], s_t[:, 0:64])
                    nc.vector.tensor_sub(rt[:, 0:64], rt[:, 0:64], m1)
                    m2 = scratch.tile([128, 64], F32, tag="m2", name="m2")
                    nc.vector.tensor_mul(rt[:, 64:128], src[:, 64:128], c_t[:, 64:128])
                    nc.vector.tensor_mul(m2, src[:, 0:64], s_t[:, 64:128])
                    nc.vector.tensor_add(rt[:, 64:128], rt[:, 64:128], m2)
                    rb = scratch.tile([128, 128], BF16, tag="rb", name="rb")
                    nc.vector.tensor_scalar_mul(rb, rt, rstd)
                    tp = psT.tile([128, 128], BF16, tag="tp", name="tp")
                    nc.tensor.transpose(tp, rb, ident)
                    dst = qT[b] if b < HP else kT
                    nc.scalar.copy(out=dst[:, it * 128:(it + 1) * 128], in_=tp)

        phase_a()
        psT_cm.__exit__(None, None, None)
        psKV_cm.__exit__(None, None, None)
        psA_cm.__exit__(None, None, None)

        # ---------------- Phases B+C fused, si-major ------------------------
        # Each open psum accumulation group needs its own bank (zero region):
        # ctx q-blocks live in separate single-buffered pools.
        with tc.tile_pool(name="psS2", bufs=1, space="PSUM") as psS2_pool, \
             tc.tile_pool(name="psC0", bufs=1, space="PSUM") as psC0, \
             tc.tile_pool(name="psC1", bufs=1, space="PSUM") as psC1, \
             tc.tile_pool(name="psT2", bufs=1, space="PSUM") as psT2, \
             tc.tile_pool(name="psO", bufs=2, space="PSUM") as psO:

            cwork = []
            psS2[0] = psS2_pool
            issue_s_mask_exp(2)

            def issue_ctx(idx):
                h, si, g = glist[idx]
                p_g = pend_p.pop(idx)
                if g == 0:
                    c0 = psC0.tile([128, 129], F32, tag="ctxq0", name="ctxq0")
                    c1 = psC1.tile([128, 129], F32, tag="ctxq1", name="ctxq1")
                    ctx_tiles[(h, si)] = (c0, c1)
                cts = ctx_tiles[(h, si)]
                for half in range(2):
                    jb = 2 * g + half
                    kk = jb - 2 * si
                    for qb in range(2):
                        if kk > qb:
                            continue  # fully-masked subblock: skip
                        nc.tensor.matmul(
                            cts[qb],
                            p_g[:, half * 256 + qb * 128:half * 256 + (qb + 1) * 128],
                            vone[jb],
                            start=(jb == 0), stop=(jb == 2 * si + qb),
                        )

            def normalize(h, si):
                cts = ctx_tiles.pop((h, si))
                for qb in range(2):
                    ct = cts[qb]
                    rden = small.tile([128, 1], F32, tag="rden", name="rden")
                    nc.vector.reciprocal(rden, ct[:, 128:129])
                    cbs = scratch.tile([128, 128], BF16, tag="cbs", name="cbs")
                    nc.vector.tensor_scalar_mul(cbs, ct[:, 0:128], rden)
                    tp = psT2.tile([128, 128], BF16, tag="tp2", name="tp2")
                    nc.tensor.transpose(tp, cbs, ident)
                    nc.vector.tensor_scalar_mul(
                        ctxT[h][:, si * 256 + qb * 128:si * 256 + (qb + 1) * 128],
                        tp, 1.0)

            def c_group(it, ot):
                o_ps = psO.tile([128, OC], F32, tag="ops", name="ops")
                for h in range(HP):
                    nc.tensor.matmul(
                        o_ps,
                        ctxT[h][:, it * 128:(it + 1) * 128],
                        wo_sb[h][:, ot * OC:(ot + 1) * OC],
                        start=(h == 0), stop=(h == HP - 1),
                    )
                o_sb = opool.tile([128, OC], F32, tag="osb", name="osb")
                hw = OC // 2
                nc.vector.tensor_scalar_mul(o_sb[:, 0:hw], o_ps[:, 0:hw], 1.0)
                nc.scalar.copy(out=o_sb[:, hw:OC], in_=o_ps[:, hw:OC])
                nc.sync.dma_start(
                    out=out_d[it * 128:(it + 1) * 128, ot * OC:(ot + 1) * OC],
                    in_=o_sb)

            for i, (h, si, g) in enumerate(glist):
                if i + LOOKAHEAD < len(glist):
                    issue_s_mask_exp(i + LOOKAHEAD)
                for _ in range(3):  # drain out-projection work into PE slack
                    if cwork:
                        c_group(*cwork.pop(0))
                issue_ctx(i)
                if g == si:  # last group of (h, si)
                    normalize(h, si)
                    if h == HP - 1:
                        cwork.extend(
                            (it, ot)
                            for it in range(2 * si, 2 * si + 2)
                            for ot in range(NO))
            while cwork:
                c_group(*cwork.pop(0))
        psS_cm.__exit__(None, None, None)

    return nc


def _prep_inputs(x, mask, cos, sin, wq, bq, wk, bk, wv, bv, wo, q_scale, k_scale):
    x2 = np.asarray(x, dtype=np.float32).reshape(T, EMB)
    # strip layout: row (it*128 + p), col (eb*128 + t) holds x[it*128+t, eb*128+p]
    xTt = x2.reshape(NT, 128, NE, 128).transpose(0, 3, 2, 1)
    xTt = np.ascontiguousarray(xTt).reshape(NT * 128, NE * 128).astype(BF)

    qs = np.asarray(q_scale, dtype=np.float32)
    ks = np.asarray(k_scale, dtype=np.float32)
    qs_rot = np.concatenate([qs[64:], qs[:64]])
    ks_rot = np.concatenate([ks[64:], ks[:64]])
    cos = np.asarray(cos, dtype=np.float32)
    sin = np.asarray(sin, dtype=np.float32)
    cosq = np.ascontiguousarray(cos * qs[None, :])
    sinq = np.ascontiguousarray(sin * qs_rot[None, :])
    cosk = np.ascontiguousarray(cos * ks[None, :])
    sink = np.ascontiguousarray(sin * ks_rot[None, :])

    jj = np.arange(128)[:, None]
    cc = np.arange(128)[None, :]
    tri = np.where(jj > cc, NEG, 0.0).astype(np.float32)
    z = np.zeros((128, 128), np.float32)
    mask256 = np.concatenate([tri, z, z, tri], axis=1)

    wq = np.asarray(wq, dtype=np.float32)
    wk = np.asarray(wk, dtype=np.float32)
    wv = np.asarray(wv, dtype=np.float32)
    wo = np.asarray(wo, dtype=np.float32)
    bq = np.asarray(bq, dtype=np.float32)
    bk = np.asarray(bk, dtype=np.float32)
    bv = np.asarray(bv, dtype=np.float32)

    in_maps = []
    for c in range(NCORES):
        wqkv_c = np.ascontiguousarray(
            np.concatenate([wq[:, c * QW:(c + 1) * QW],
                            wk[:, c * D:(c + 1) * D],
                            wv[:, c * D:(c + 1) * D]], axis=1)).astype(BF)
        wo_c = np.ascontiguousarray(wo[c * QW:(c + 1) * QW, :]).astype(BF)
        brow_c = np.concatenate(
            [bq[c * QW:(c + 1) * QW], bk[c * D:(c + 1) * D],
             bv[c * D:(c + 1) * D]]).reshape(1, WW).astype(BF)
        in_maps.append({
            "xT": xTt,
            "wqkv": wqkv_c,
            "wo": wo_c,
            "cosq": cosq, "sinq": sinq, "cosk": cosk, "sink": sink,
            "mask256": mask256,
            "ident": np.eye(128, dtype=BF),
            "brow": brow_c,
        })
    return in_maps


def _get_program(zero_bias=True):
    key = ("nc", zero_bias)
    if key not in _prog_cache:
        nc = _build_program(zero_bias)
        if not nc.is_finalized():
            nc.finalize()
        _prog_cache[key] = nc
    return _prog_cache[key]


def kernel(**inputs):
    zb = not (np.any(inputs["bq"]) or np.any(inputs["bk"])
              or np.any(inputs["bv"]))
    in_maps = _prep_inputs(**inputs)
    nc = _get_program(bool(zb))
    res = run_bass_kernel_spmd(nc, in_maps, list(range(NCORES)))
    out = np.zeros((T, EMB), dtype=np.float32)
    for r in res.results:
        out += np.asarray(r["out"], dtype=np.float32)
    return out.reshape(1, T, EMB)


# revision 10
# speedup vs baseline: 1.0287x; 1.0245x over previous
"""GQA attention kernel for Trainium2, tensor-parallel over heads on 8 cores.

Problem: B=1, T=2048, EMB=4096, H=32 query heads, G=8 KV groups, D=128.
Reference: q/k/v projections -> per-head RMS norm (q,k) -> RoPE (q,k) ->
causal GQA attention -> out projection.

Sharding: core c owns query heads [4c, 4c+4) and KV group c.  Each core
computes a partial output x @ W_o_rows for its heads; host sums the 8
partials (the all-reduce of the module's TP scheme).

Design notes:
  - softmax denominator is FREE: ctx matmuls use P-block [k,128q] as the
    stationary operand and [v | ones] (129 moving cols) as the moving one;
    den accumulates in psum col 128 of the same accumulation group.
  - normalization is a per-partition tensor_scalar multiply (q is the
    partition dim of the ctx psum tile) -- no partition broadcast needed.
  - attention runs in 256-query slices; causal sub-block skipping: the
    ctx matmul (qb, jb) is only issued when jb <= 2*si+qb, so the
    fully-masked P regions are never read; the mask is a [128,128]
    triangle added only on diagonal subblocks (post-matmul on DVE --
    PE accumulate does NOT see DVE-prewritten psum on real HW).
  - qkv biases fold into the projection accumulation as a K=1 matmul
    (ones[1,128] x brow[1,768]); when the biases are all zero (the
    common case) a specialized program variant skips them entirely.
  - each open psum accumulation group needs its own bank (zero region):
    8 banks = psS(2)+psS2(1) S-groups, ctxq0(1), ctxq1(1), psT2(1),
    psO(2); S-groups rotate 3-deep across psS/psS2.
  - phases: A (projections+rms+rope+transpose), then B (attention) and
    C (out projection) fused: C ot-groups are drained into the B stream
    (2 per group-iteration) so PE fills ACT-bound exp slack; the first
    B S/exp groups are issued during the last A tile.
  - the SP queue issues DMAs serially (blocking for the transfer), so
    DMA program order == arrival order: wqkv e0-3, x strip 0, e4-15,
    x strip 1, rope tables, consts, remaining wqkv; wo/mask issued at
    A-tile 2; exp table loaded once (Square/Copy live in every set).
    The last A tile copies q/kv psum to SBUF so the A psum banks (and
    with them the B pools) free before its rms/rope chain drains.
"""

import numpy as np
import ml_dtypes
from contextlib import ExitStack

import concourse.bacc as bacc
import concourse.mybir as mybir
from concourse.tile import TileContext
from concourse.bass_utils import run_bass_kernel_spmd

EMB, H, G, D, T = 4096, 32, 8, 128, 2048
EPS = 1e-6
NCORES = 8
HP = H // NCORES          # 4 query heads per core
NT = T // 128             # 16 t-tiles
NE = EMB // 128           # 32 e-tiles
QW = HP * D               # 512 = q width per core
KVW = 2 * D               # 256 = k|v width per core
WW = QW + KVW             # 768 = packed qkv width per core
SM_SCALE = 1.0 / float(np.sqrt(D))
NEG = -1e9
OC = 512                  # phase C output column tile width
NO = EMB // OC            # 16 output column tiles

F32 = mybir.dt.float32
BF16 = mybir.dt.bfloat16
BF = ml_dtypes.bfloat16

_prog_cache = {}


def _build_program(zero_bias):
    nc = bacc.Bacc()

    xT_d = nc.declare_dram_parameter("xT", [NT * 128, NE * 128], BF16, isOutput=False)
    wqkv_d = nc.declare_dram_parameter("wqkv", [NE * 128, WW], BF16, isOutput=False)
    wo_d = nc.declare_dram_parameter("wo", [HP * 128, EMB], BF16, isOutput=False)
    cq_d = nc.declare_dram_parameter("cosq", [NT * 128, 128], F32, isOutput=False)
    sq_d = nc.declare_dram_parameter("sinq", [NT * 128, 128], F32, isOutput=False)
    ck_d = nc.declare_dram_parameter("cosk", [NT * 128, 128], F32, isOutput=False)
    sk_d = nc.declare_dram_parameter("sink", [NT * 128, 128], F32, isOutput=False)
    mask_d = nc.declare_dram_parameter("mask256", [128, 512], F32, isOutput=False)
    ident_d = nc.declare_dram_parameter("ident", [128, 128], BF16, isOutput=False)
    brow_d = (None if zero_bias else
              nc.declare_dram_parameter("brow", [1, WW], BF16, isOutput=False))
    out_d = nc.declare_dram_parameter("out", [T, EMB], F32, isOutput=True)

    with TileContext(nc) as tc, ExitStack() as ctx:
        consts = ctx.enter_context(tc.tile_pool(name="consts", bufs=1))
        wpool = ctx.enter_context(tc.tile_pool(name="wpool", bufs=1))
        xpool = ctx.enter_context(tc.tile_pool(name="xpool", bufs=2))
        cspool = ctx.enter_context(tc.tile_pool(name="cspool", bufs=2))
        scratch = ctx.enter_context(tc.tile_pool(name="scratch", bufs=8))
        small = ctx.enter_context(tc.tile_pool(name="small", bufs=12))
        resid = ctx.enter_context(tc.tile_pool(name="resid", bufs=1))
        ppool = ctx.enter_context(tc.tile_pool(name="ppool", bufs=6))
        opool = ctx.enter_context(tc.tile_pool(name="opool", bufs=6))

        # ---- early DMAs: the SP queue issues serially, so order = arrival.
        # First-needed first: wqkv e=0-3, x strip 0, more wqkv, x strip 1,
        # rope tables for tiles 0-1, brow/ident, remaining wqkv strips.
        TILE_ORDER = list(range(NT))
        wqkv_sb = [wpool.tile([128, WW], BF16, tag=f"wqkv{e}", name=f"wqkv{e}")
                   for e in range(NE)]

        def _wq_dma(e):
            nc.sync.dma_start(out=wqkv_sb[e],
                              in_=wqkv_d[e * 128:(e + 1) * 128, :])

        for e in range(4):
            _wq_dma(e)
        # hoisted strips go out on the otherwise-idle ACT DMA queue, rope
        # tables + ident on the DVE queue -- all three queues transfer in
        # parallel with the SP weight stream during startup
        xs_hoist = {}
        for hidx, it in enumerate(TILE_ORDER[:2]):
            xs = xpool.tile([128, NE * 128], BF16, tag="xstrip", name="xstrip")
            nc.scalar.dma_start(out=xs, in_=xT_d[it * 128:(it + 1) * 128, :])
            xs_hoist[it] = xs
        cs_hoist = {}
        for it in TILE_ORDER[:2]:
            tiles = []
            for nm, d in (("cq", cq_d), ("sq", sq_d), ("ck", ck_d), ("sk", sk_d)):
                t = cspool.tile([128, 128], F32, tag=nm, name=nm)
                nc.gpsimd.dma_start(out=t, in_=d[it * 128:(it + 1) * 128, :])
                tiles.append(t)
            cs_hoist[it] = tiles
        ident = consts.tile([128, 128], BF16, tag="ident", name="ident")
        nc.gpsimd.dma_start(out=ident, in_=ident_d[:, :])
        for e in range(4, 11):
            _wq_dma(e)
        eps_t = consts.tile([128, 1], F32, tag="eps", name="eps")
        nc.vector.memset(eps_t, EPS)
        # pre-warm the Sqrt act-func set during the DMA dead time so the
        # 1.3us table loads stay off the first tile's rms chain (Square and
        # Copy live in the same set; only Exp needs another load, at B start)
        warm_t = consts.tile([128, 1], F32, tag="warm", name="warm")
        nc.scalar.activation(out=warm_t, in_=eps_t,
                             func=mybir.ActivationFunctionType.Sqrt)
        # PE p-state warm-up: keep the PE continuously busy on dummy matmuls
        # through the initial DMA wait so the clock is fully ramped when the
        # first projection matmul's inputs land. A late ramp costs ~2.5us;
        # if the DMAs beat the dummies the loss is bounded by one dummy.
        dmA = consts.tile([128, 128], BF16, tag="dmA", name="dmA")
        nc.vector.memset(dmA, 0.0)
        dmB = consts.tile([128, 512], BF16, tag="dmB", name="dmB")
        nc.vector.memset(dmB, 0.0)
        if not zero_bias:
            brow = consts.tile([1, WW], BF16, tag="brow", name="brow")
            nc.sync.dma_start(out=brow, in_=brow_d[:, :])
            ones1 = consts.tile([1, 128], BF16, tag="ones1", name="ones1")
            nc.vector.memset(ones1, 1.0)
        for e in range(11, NE):
            _wq_dma(e)
        mask_sb = consts.tile([128, 512], F32, tag="mask", name="mask")
        wo_sb = [wpool.tile([128, EMB], BF16, tag=f"wo{h}", name=f"wo{h}")
                 for h in range(HP)]

        # resident activations
        qT = [resid.tile([128, T], BF16, tag=f"qT{h}", name=f"qT{h}") for h in range(HP)]
        kT = resid.tile([128, T], BF16, tag="kT", name="kT")
        vone = [resid.tile([128, 129], BF16, tag=f"v{j}", name=f"v{j}") for j in range(NT)]
        ctxT = [resid.tile([128, T], BF16, tag=f"ctxT{h}", name=f"ctxT{h}") for h in range(HP)]
        for j in range(NT):
            nc.vector.memset(vone[j][:, 128:129], 1.0)

        # ---------------- Phases ------------------------------------------
        # A: projections + rms + rope + transpose.  B: attention (256-query
        # slices, P-stationary ctx matmuls with a ones column for the free
        # softmax denominator).  C: out projection, interleaved into B's
        # stream so PE fills ACT-bound slack.  The first B groups are issued
        # during late phase A (psS opens alongside the A pools).
        psS_cm = tc.tile_pool(name="psS", bufs=2, space="PSUM")
        psA_cm = tc.tile_pool(name="psA", bufs=2, space="PSUM")
        psKV_cm = tc.tile_pool(name="psKV", bufs=2, space="PSUM")
        psT_cm = tc.tile_pool(name="psT", bufs=2, space="PSUM")
        psS = psS_cm.__enter__()
        psA = psA_cm.__enter__()
        psKV = psKV_cm.__enter__()
        psT = psT_cm.__enter__()

        NSI = 8
        glist = []
        for si in range(NSI):
            for h in range(HP):
                glist.extend((h, si, g) for g in range(si + 1))
        pend_p = {}
        ctx_tiles = {}
        psS2 = [None]

        def issue_s_mask_exp(idx):
            h, si, g = glist[idx]
            pool = psS if idx % 3 < 2 else psS2[0]
            s_g = pool.tile([128, 512], F32, tag="sg", name="sg")
            for half in range(2):
                jb = 2 * g + half
                nc.tensor.matmul(
                    s_g[:, half * 256:(half + 1) * 256],
                    kT[:, jb * 128:(jb + 1) * 128],
                    qT[h][:, si * 256:(si + 1) * 256],
                    start=True, stop=True,
                )
            for half in range(2):
                kk = 2 * g + half - 2 * si
                if kk >= 0:  # diagonal: triangular mask on its subblock
                    off = half * 256 + kk * 128
                    nc.vector.tensor_add(
                        s_g[:, off:off + 128], s_g[:, off:off + 128],
                        mask_sb[:, kk * 256 + kk * 128:kk * 256 + (kk + 1) * 128])
            p_g = ppool.tile([128, 512], BF16, tag="pt", name="pt")
            nc.scalar.activation(
                out=p_g, in_=s_g,
                func=mybir.ActivationFunctionType.Exp,
                scale=SM_SCALE,
            )
            pend_p[idx] = p_g

        LOOKAHEAD = 3

        def phase_a():
            for w in range(26):
                wps = psS.tile([128, 512], F32, tag="sg", name="sg")
                nc.tensor.matmul(wps, dmA, dmB, start=True, stop=True)
            for idx, it in enumerate(TILE_ORDER):
                q_ps = psA.tile([128, QW], F32, tag="aq", name="aq")
                kv_ps = psKV.tile([128, KVW], F32, tag="akv", name="akv")
                if it in xs_hoist:
                    xstrip = xs_hoist.pop(it)
                else:
                    xstrip = xpool.tile([128, NE * 128], BF16, tag="xstrip",
                                        name="xstrip")
                    nc.sync.dma_start(out=xstrip, in_=xT_d[it * 128:(it + 1) * 128, :])
                for e in range(NE):
                    xt = xstrip[:, e * 128:(e + 1) * 128]
                    last = zero_bias and e == NE - 1
                    nc.tensor.matmul(kv_ps, xt, wqkv_sb[e][:, QW:WW],
                                     start=(e == 0), stop=last)
                for e in range(NE):
                    xt = xstrip[:, e * 128:(e + 1) * 128]
                    last = zero_bias and e == NE - 1
                    nc.tensor.matmul(q_ps, xt, wqkv_sb[e][:, 0:QW],
                                     start=(e == 0), stop=last)
                if not zero_bias:
                    # bias add fused into the accumulation as a K=1 matmul
                    nc.tensor.matmul(q_ps, ones1, brow[:, 0:QW],
                                     start=False, stop=True)
                    nc.tensor.matmul(kv_ps, ones1, brow[:, QW:WW],
                                     start=False, stop=True)
                if idx == NT - 1:
                    # pre-fill the attention pipeline; si=0 needs only
                    # qT/kT t-tiles 0-1, ready long ago
                    issue_s_mask_exp(0)
                    issue_s_mask_exp(1)

                if it in cs_hoist:
                    cq, sq, ck, sk = cs_hoist.pop(it)
                else:
                    cq = cspool.tile([128, 128], F32, tag="cq", name="cq")
                    nc.gpsimd.dma_start(out=cq, in_=cq_d[it * 128:(it + 1) * 128, :])
                    sq = cspool.tile([128, 128], F32, tag="sq", name="sq")
                    nc.gpsimd.dma_start(out=sq, in_=sq_d[it * 128:(it + 1) * 128, :])
                    ck = cspool.tile([128, 128], F32, tag="ck", name="ck")
                    nc.gpsimd.dma_start(out=ck, in_=ck_d[it * 128:(it + 1) * 128, :])
                    sk = cspool.tile([128, 128], F32, tag="sk", name="sk")
                    nc.gpsimd.dma_start(out=sk, in_=sk_d[it * 128:(it + 1) * 128, :])
                if idx == 2:
                    nc.sync.dma_start(out=mask_sb, in_=mask_d[:, :])
                    for hh in range(HP):
                        nc.sync.dma_start(
                            out=wo_sb[hh], in_=wo_d[hh * 128:(hh + 1) * 128, :])

                if idx == NT - 1:
                    # free the A psum banks early: the B pools allocate them
                    qsb = scratch.tile([128, QW], F32, tag="qsb", name="qsb")
                    nc.scalar.copy(out=qsb, in_=q_ps)
                    kvsb = scratch.tile([128, KVW], F32, tag="kvsb", name="kvsb")
                    nc.scalar.copy(out=kvsb, in_=kv_ps)
                    q_src, kv_src = qsb, kvsb
                else:
                    q_src, kv_src = q_ps, kv_ps
                nc.vector.tensor_scalar_mul(
                    vone[it][:, 0:128], kv_src[:, 128:256], 1.0)
                for b in (HP, 0, 1, 2, 3):  # k first, then q heads
                    if b < HP:
                        src = q_src[:, b * 128:(b + 1) * 128]
                        c_t, s_t = cq[:, :], sq[:, :]
                    else:
                        src = kv_src[:, 0:128]
                        c_t, s_t = ck[:, :], sk[:, :]
                    sqout = scratch.tile([128, 128], F32, tag="sqout", name="sqout")
                    sqacc = small.tile([128, 1], F32, tag="sqacc", name="sqacc")
                    nc.scalar.activation(
                        out=sqout, in_=src,
                        func=mybir.ActivationFunctionType.Square,
                        accum_out=sqacc,
                    )
                    rstd = small.tile([128, 1], F32, tag="rstd", name="rstd")
                    nc.scalar.activation(
                        out=rstd, in_=sqacc,
                        func=mybir.ActivationFunctionType.Sqrt,
                        bias=eps_t, scale=1.0 / D,
                    )
                    nc.vector.reciprocal(rstd, rstd)
                    # rope: out1 = x1*c1 - x2*s1 ; out2 = x2*c2 + x1*s2
                    rt = scratch.tile([128, 128], F32, tag="rt", name="rt")
                    m1 = scratch.tile([128, 64], F32, tag="m1", name="m1")
                    nc.vector.tensor_mul(rt[:, 0:64], src[:, 0:64], c_t[:, 0:64])
                    nc.vector.tensor_mul(m1, src[:, 64:128], s_t[:, 0:64])
                    nc.vector.tensor_sub(rt[:, 0:64], rt[:, 0:64], m1)
                    m2 = scratch.tile([128, 64], F32, tag="m2", name="m2")
                    nc.vector.tensor_mul(rt[:, 64:128], src[:, 64:128], c_t[:, 64:128])
                    nc.vector.tensor_mul(m2, src[:, 0:64], s_t[:, 64:128])
                    nc.vector.tensor_add(rt[:, 64:128], rt[:, 64:128], m2)
                    rb = scratch.tile([128, 128], BF16, tag="rb", name="rb")
                    nc.vector.tensor_scalar_mul(rb, rt, rstd)
                    tp = psT.tile([128, 128], BF16, tag="tp", name="tp")
                    nc.tensor.transpose(tp, rb, ident)
                    dst = qT[b] if b < HP else kT
                    nc.scalar.copy(out=dst[:, it * 128:(it + 1) * 128], in_=tp)

        phase_a()
        psT_cm.__exit__(None, None, None)
        psKV_cm.__exit__(None, None, None)
        psA_cm.__exit__(None, None, None)

        # ---------------- Phases B+C fused, si-major ------------------------
        # Each open psum accumulation group needs its own bank (zero region):
        # ctx q-blocks live in separate single-buffered pools.
        with tc.tile_pool(name="psS2", bufs=1, space="PSUM") as psS2_pool, \
             tc.tile_pool(name="psC0", bufs=1, space="PSUM") as psC0, \
             tc.tile_pool(name="psC1", bufs=1, space="PSUM") as psC1, \
             tc.tile_pool(name="psT2", bufs=1, space="PSUM") as psT2, \
             tc.tile_pool(name="psO", bufs=2, space="PSUM") as psO:

            cwork = []
            psS2[0] = psS2_pool
            issue_s_mask_exp(2)

            def issue_ctx(idx):
                h, si, g = glist[idx]
                p_g = pend_p.pop(idx)
                if g == 0:
                    c0 = psC0.tile([128, 129], F32, tag="ctxq0", name="ctxq0")
                    c1 = psC1.tile([128, 129], F32, tag="ctxq1", name="ctxq1")
                    ctx_tiles[(h, si)] = (c0, c1)
                cts = ctx_tiles[(h, si)]
                for half in range(2):
                    jb = 2 * g + half
                    kk = jb - 2 * si
                    for qb in range(2):
                        if kk > qb:
                            continue  # fully-masked subblock: skip
                        nc.tensor.matmul(
                            cts[qb],
                            p_g[:, half * 256 + qb * 128:half * 256 + (qb + 1) * 128],
                            vone[jb],
                            start=(jb == 0), stop=(jb == 2 * si + qb),
                        )

            def normalize(h, si):
                cts = ctx_tiles.pop((h, si))
                for qb in range(2):
                    ct = cts[qb]
                    rden = small.tile([128, 1], F32, tag="rden", name="rden")
                    nc.vector.reciprocal(rden, ct[:, 128:129])
                    cbs = scratch.tile([128, 128], BF16, tag="cbs", name="cbs")
                    nc.vector.tensor_scalar_mul(cbs, ct[:, 0:128], rden)
                    tp = psT2.tile([128, 128], BF16, tag="tp2", name="tp2")
                    nc.tensor.transpose(tp, cbs, ident)
                    nc.vector.tensor_scalar_mul(
                        ctxT[h][:, si * 256 + qb * 128:si * 256 + (qb + 1) * 128],
                        tp, 1.0)

            def c_group(it, ot):
                o_ps = psO.tile([128, OC], F32, tag="ops", name="ops")
                for h in range(HP):
                    nc.tensor.matmul(
                        o_ps,
                        ctxT[h][:, it * 128:(it + 1) * 128],
                        wo_sb[h][:, ot * OC:(ot + 1) * OC],
                        start=(h == 0), stop=(h == HP - 1),
                    )
                o_sb = opool.tile([128, OC], F32, tag="osb", name="osb")
                hw = OC // 2
                nc.vector.tensor_scalar_mul(o_sb[:, 0:hw], o_ps[:, 0:hw], 1.0)
                nc.scalar.copy(out=o_sb[:, hw:OC], in_=o_ps[:, hw:OC])
                eng = nc.sync if (it + ot) % 2 == 0 else nc.gpsimd
                eng.dma_start(
                    out=out_d[it * 128:(it + 1) * 128, ot * OC:(ot + 1) * OC],
                    in_=o_sb)

            for i, (h, si, g) in enumerate(glist):
                if i + LOOKAHEAD < len(glist):
                    issue_s_mask_exp(i + LOOKAHEAD)
                for _ in range(3):  # drain out-projection work into PE slack
                    if cwork:
                        c_group(*cwork.pop(0))
                issue_ctx(i)
                if g == si:  # last group of (h, si)
                    normalize(h, si)
                    if h == HP - 1:
                        cwork.extend(
                            (it, ot)
                            for it in range(2 * si, 2 * si + 2)
                            for ot in range(NO))
            while cwork:
                c_group(*cwork.pop(0))
        psS_cm.__exit__(None, None, None)

    return nc


def _prep_inputs(x, mask, cos, sin, wq, bq, wk, bk, wv, bv, wo, q_scale, k_scale):
    x2 = np.asarray(x, dtype=np.float32).reshape(T, EMB)
    # strip layout: row (it*128 + p), col (eb*128 + t) holds x[it*128+t, eb*128+p]
    xTt = x2.reshape(NT, 128, NE, 128).transpose(0, 3, 2, 1)
    xTt = np.ascontiguousarray(xTt).reshape(NT * 128, NE * 128).astype(BF)

    qs = np.asarray(q_scale, dtype=np.float32)
    ks = np.asarray(k_scale, dtype=np.float32)
    qs_rot = np.concatenate([qs[64:], qs[:64]])
    ks_rot = np.concatenate([ks[64:], ks[:64]])
    cos = np.asarray(cos, dtype=np.float32)
    sin = np.asarray(sin, dtype=np.float32)
    cosq = np.ascontiguousarray(cos * qs[None, :])
    sinq = np.ascontiguousarray(sin * qs_rot[None, :])
    cosk = np.ascontiguousarray(cos * ks[None, :])
    sink = np.ascontiguousarray(sin * ks_rot[None, :])

    jj = np.arange(128)[:, None]
    cc = np.arange(128)[None, :]
    tri = np.where(jj > cc, NEG, 0.0).astype(np.float32)
    z = np.zeros((128, 128), np.float32)
    mask256 = np.concatenate([tri, z, z, tri], axis=1)

    wq = np.asarray(wq, dtype=np.float32)
    wk = np.asarray(wk, dtype=np.float32)
    wv = np.asarray(wv, dtype=np.float32)
    wo = np.asarray(wo, dtype=np.float32)
    bq = np.asarray(bq, dtype=np.float32)
    bk = np.asarray(bk, dtype=np.float32)
    bv = np.asarray(bv, dtype=np.float32)

    in_maps = []
    for c in range(NCORES):
        wqkv_c = np.ascontiguousarray(
            np.concatenate([wq[:, c * QW:(c + 1) * QW],
                            wk[:, c * D:(c + 1) * D],
                            wv[:, c * D:(c + 1) * D]], axis=1)).astype(BF)
        wo_c = np.ascontiguousarray(wo[c * QW:(c + 1) * QW, :]).astype(BF)
        brow_c = np.concatenate(
            [bq[c * QW:(c + 1) * QW], bk[c * D:(c + 1) * D],
             bv[c * D:(c + 1) * D]]).reshape(1, WW).astype(BF)
        in_maps.append({
            "xT": xTt,
            "wqkv": wqkv_c,
            "wo": wo_c,
            "cosq": cosq, "sinq": sinq, "cosk": cosk, "sink": sink,
            "mask256": mask256,
            "ident": np.eye(128, dtype=BF),
            "brow": brow_c,
        })
    return in_maps


def _get_program(zero_bias=True):
    key = ("nc", zero_bias)
    if key not in _prog_cache:
        nc = _build_program(zero_bias)
        if not nc.is_finalized():
            nc.finalize()
        _prog_cache[key] = nc
    return _prog_cache[key]


def kernel(**inputs):
    zb = not (np.any(inputs["bq"]) or np.any(inputs["bk"])
              or np.any(inputs["bv"]))
    in_maps = _prep_inputs(**inputs)
    nc = _get_program(bool(zb))
    res = run_bass_kernel_spmd(nc, in_maps, list(range(NCORES)))
    out = np.zeros((T, EMB), dtype=np.float32)
    for r in res.results:
        out += np.asarray(r["out"], dtype=np.float32)
    return out.reshape(1, T, EMB)


# revision 11
# speedup vs baseline: 1.0323x; 1.0035x over previous
"""GQA attention kernel for Trainium2, tensor-parallel over heads on 8 cores.

Problem: B=1, T=2048, EMB=4096, H=32 query heads, G=8 KV groups, D=128.
Reference: q/k/v projections -> per-head RMS norm (q,k) -> RoPE (q,k) ->
causal GQA attention -> out projection.

Sharding: core c owns query heads [4c, 4c+4) and KV group c.  Each core
computes a partial output x @ W_o_rows for its heads; host sums the 8
partials (the all-reduce of the module's TP scheme).

Design notes:
  - softmax denominator is FREE: ctx matmuls use P-block [k,128q] as the
    stationary operand and [v | ones] (129 moving cols) as the moving one;
    den accumulates in psum col 128 of the same accumulation group.
  - normalization is a per-partition tensor_scalar multiply (q is the
    partition dim of the ctx psum tile) -- no partition broadcast needed.
  - attention runs in 256-query slices; causal sub-block skipping: the
    ctx matmul (qb, jb) is only issued when jb <= 2*si+qb, so the
    fully-masked P regions are never read; the mask is a [128,128]
    triangle added only on diagonal subblocks (post-matmul on DVE --
    PE accumulate does NOT see DVE-prewritten psum on real HW).
  - qkv biases fold into the projection accumulation as a K=1 matmul
    (ones[1,128] x brow[1,768]); when the biases are all zero (the
    common case) a specialized program variant skips them entirely.
  - each open psum accumulation group needs its own bank (zero region):
    8 banks = psS(2)+psS2(1) S-groups, ctxq0(1), ctxq1(1), psT2(1),
    psO(2); S-groups rotate 3-deep across psS/psS2.
  - phases: A (projections+rms+rope+transpose), then B (attention) and
    C (out projection) fused: C ot-groups are drained into the B stream
    (2 per group-iteration) so PE fills ACT-bound exp slack; the first
    B S/exp groups are issued during the last A tile.
  - the SP queue issues DMAs serially (blocking for the transfer), so
    DMA program order == arrival order: wqkv e0-3, x strip 0, e4-15,
    x strip 1, rope tables, consts, remaining wqkv; wo/mask issued at
    A-tile 2; exp table loaded once (Square/Copy live in every set).
    The last A tile copies q/kv psum to SBUF so the A psum banks (and
    with them the B pools) free before its rms/rope chain drains.
"""

import numpy as np
import ml_dtypes
from contextlib import ExitStack

import concourse.bacc as bacc
import concourse.mybir as mybir
from concourse.tile import TileContext
from concourse.bass_utils import run_bass_kernel_spmd

EMB, H, G, D, T = 4096, 32, 8, 128, 2048
EPS = 1e-6
NCORES = 8
HP = H // NCORES          # 4 query heads per core
NT = T // 128             # 16 t-tiles
NE = EMB // 128           # 32 e-tiles
QW = HP * D               # 512 = q width per core
KVW = 2 * D               # 256 = k|v width per core
WW = QW + KVW             # 768 = packed qkv width per core
SM_SCALE = 1.0 / float(np.sqrt(D))
NEG = -1e9
OC = 512                  # phase C output column tile width
NO = EMB // OC            # 16 output column tiles

F32 = mybir.dt.float32
BF16 = mybir.dt.bfloat16
BF = ml_dtypes.bfloat16

_prog_cache = {}


def _build_program(zero_bias):
    nc = bacc.Bacc()

    xT_d = nc.declare_dram_parameter("xT", [NT * 128, NE * 128], BF16, isOutput=False)
    wqkv_d = nc.declare_dram_parameter("wqkv", [NE * 128, WW], BF16, isOutput=False)
    wo_d = nc.declare_dram_parameter("wo", [HP * 128, EMB], BF16, isOutput=False)
    cq_d = nc.declare_dram_parameter("cosq", [NT * 128, 128], F32, isOutput=False)
    sq_d = nc.declare_dram_parameter("sinq", [NT * 128, 128], F32, isOutput=False)
    ck_d = nc.declare_dram_parameter("cosk", [NT * 128, 128], F32, isOutput=False)
    sk_d = nc.declare_dram_parameter("sink", [NT * 128, 128], F32, isOutput=False)
    mask_d = nc.declare_dram_parameter("mask256", [128, 512], F32, isOutput=False)
    ident_d = nc.declare_dram_parameter("ident", [128, 128], BF16, isOutput=False)
    brow_d = (None if zero_bias else
              nc.declare_dram_parameter("brow", [1, WW], BF16, isOutput=False))
    out_d = nc.declare_dram_parameter("out", [T, EMB], F32, isOutput=True)

    with TileContext(nc) as tc, ExitStack() as ctx:
        consts = ctx.enter_context(tc.tile_pool(name="consts", bufs=1))
        wpool = ctx.enter_context(tc.tile_pool(name="wpool", bufs=1))
        xpool = ctx.enter_context(tc.tile_pool(name="xpool", bufs=2))
        cspool = ctx.enter_context(tc.tile_pool(name="cspool", bufs=2))
        scratch = ctx.enter_context(tc.tile_pool(name="scratch", bufs=8))
        small = ctx.enter_context(tc.tile_pool(name="small", bufs=12))
        resid = ctx.enter_context(tc.tile_pool(name="resid", bufs=1))
        ppool = ctx.enter_context(tc.tile_pool(name="ppool", bufs=6))
        opool = ctx.enter_context(tc.tile_pool(name="opool", bufs=6))

        # ---- early DMAs: the SP queue issues serially, so order = arrival.
        # First-needed first: wqkv e=0-3, x strip 0, more wqkv, x strip 1,
        # rope tables for tiles 0-1, brow/ident, remaining wqkv strips.
        TILE_ORDER = list(range(NT))
        wqkv_sb = [wpool.tile([128, WW], BF16, tag=f"wqkv{e}", name=f"wqkv{e}")
                   for e in range(NE)]

        def _wq_dma(e):
            nc.sync.dma_start(out=wqkv_sb[e],
                              in_=wqkv_d[e * 128:(e + 1) * 128, :])

        for e in range(4):
            _wq_dma(e)
        # hoisted strips go out on the otherwise-idle ACT DMA queue, rope
        # tables + ident on the DVE queue -- all three queues transfer in
        # parallel with the SP weight stream during startup
        xs_hoist = {}
        for hidx, it in enumerate(TILE_ORDER[:2]):
            xs = xpool.tile([128, NE * 128], BF16, tag="xstrip", name="xstrip")
            nc.scalar.dma_start(out=xs, in_=xT_d[it * 128:(it + 1) * 128, :])
            xs_hoist[it] = xs
        cs_hoist = {}
        for it in TILE_ORDER[:2]:
            tiles = []
            for nm, d in (("cq", cq_d), ("sq", sq_d), ("ck", ck_d), ("sk", sk_d)):
                t = cspool.tile([128, 128], F32, tag=nm, name=nm)
                nc.gpsimd.dma_start(out=t, in_=d[it * 128:(it + 1) * 128, :])
                tiles.append(t)
            cs_hoist[it] = tiles
        ident = consts.tile([128, 128], BF16, tag="ident", name="ident")
        nc.gpsimd.dma_start(out=ident, in_=ident_d[:, :])
        for e in range(4, 11):
            _wq_dma(e)
        eps_t = consts.tile([128, 1], F32, tag="eps", name="eps")
        nc.vector.memset(eps_t, EPS)
        # pre-warm the Sqrt act-func set during the DMA dead time so the
        # 1.3us table loads stay off the first tile's rms chain (Square and
        # Copy live in the same set; only Exp needs another load, at B start)
        warm_t = consts.tile([128, 1], F32, tag="warm", name="warm")
        nc.scalar.activation(out=warm_t, in_=eps_t,
                             func=mybir.ActivationFunctionType.Sqrt)
        # PE p-state warm-up: keep the PE continuously busy on dummy matmuls
        # through the initial DMA wait so the clock is fully ramped when the
        # first projection matmul's inputs land. A late ramp costs ~2.5us;
        # if the DMAs beat the dummies the loss is bounded by one dummy.
        dmA = consts.tile([128, 128], BF16, tag="dmA", name="dmA")
        nc.vector.memset(dmA, 0.0)
        dmB = consts.tile([128, 512], BF16, tag="dmB", name="dmB")
        nc.vector.memset(dmB, 0.0)
        if not zero_bias:
            brow = consts.tile([1, WW], BF16, tag="brow", name="brow")
            nc.sync.dma_start(out=brow, in_=brow_d[:, :])
            ones1 = consts.tile([1, 128], BF16, tag="ones1", name="ones1")
            nc.vector.memset(ones1, 1.0)
        for e in range(11, NE):
            _wq_dma(e)
        mask_sb = consts.tile([128, 512], F32, tag="mask", name="mask")
        wo_sb = [wpool.tile([128, EMB], BF16, tag=f"wo{h}", name=f"wo{h}")
                 for h in range(HP)]

        # resident activations
        qT = [resid.tile([128, T], BF16, tag=f"qT{h}", name=f"qT{h}") for h in range(HP)]
        kT = resid.tile([128, T], BF16, tag="kT", name="kT")
        vone = [resid.tile([128, 129], BF16, tag=f"v{j}", name=f"v{j}") for j in range(NT)]
        ctxT = [resid.tile([128, T], BF16, tag=f"ctxT{h}", name=f"ctxT{h}") for h in range(HP)]
        for j in range(NT):
            nc.vector.memset(vone[j][:, 128:129], 1.0)

        # ---------------- Phases ------------------------------------------
        # A: projections + rms + rope + transpose.  B: attention (256-query
        # slices, P-stationary ctx matmuls with a ones column for the free
        # softmax denominator).  C: out projection, interleaved into B's
        # stream so PE fills ACT-bound slack.  The first B groups are issued
        # during late phase A (psS opens alongside the A pools).
        psS_cm = tc.tile_pool(name="psS", bufs=2, space="PSUM")
        psS2_cm = tc.tile_pool(name="psS2", bufs=1, space="PSUM")
        psA_cm = tc.tile_pool(name="psA", bufs=2, space="PSUM")
        psKV_cm = tc.tile_pool(name="psKV", bufs=2, space="PSUM")
        psT_cm = tc.tile_pool(name="psT", bufs=1, space="PSUM")
        psS = psS_cm.__enter__()
        psS2_pool_early = psS2_cm.__enter__()
        psA = psA_cm.__enter__()
        psKV = psKV_cm.__enter__()
        psT = psT_cm.__enter__()

        NSI = 8
        glist = []
        for si in range(NSI):
            for h in range(HP):
                glist.extend((h, si, g) for g in range(si + 1))
        pend_p = {}
        ctx_tiles = {}
        psS2 = [None]
        psS2[0] = psS2_pool_early

        def issue_s_mask_exp(idx):
            h, si, g = glist[idx]
            pool = psS if idx % 3 < 2 else psS2[0]
            s_g = pool.tile([128, 512], F32, tag="sg", name="sg")
            for half in range(2):
                jb = 2 * g + half
                nc.tensor.matmul(
                    s_g[:, half * 256:(half + 1) * 256],
                    kT[:, jb * 128:(jb + 1) * 128],
                    qT[h][:, si * 256:(si + 1) * 256],
                    start=True, stop=True,
                )
            for half in range(2):
                kk = 2 * g + half - 2 * si
                if kk >= 0:  # diagonal: triangular mask on its subblock
                    off = half * 256 + kk * 128
                    nc.vector.tensor_add(
                        s_g[:, off:off + 128], s_g[:, off:off + 128],
                        mask_sb[:, kk * 256 + kk * 128:kk * 256 + (kk + 1) * 128])
            p_g = ppool.tile([128, 512], BF16, tag="pt", name="pt")
            nc.scalar.activation(
                out=p_g, in_=s_g,
                func=mybir.ActivationFunctionType.Exp,
                scale=SM_SCALE,
            )
            pend_p[idx] = p_g

        LOOKAHEAD = 3

        def phase_a():
            for w in range(26):
                wps = psS.tile([128, 512], F32, tag="sg", name="sg")
                nc.tensor.matmul(wps, dmA, dmB, start=True, stop=True)
            for idx, it in enumerate(TILE_ORDER):
                q_ps = psA.tile([128, QW], F32, tag="aq", name="aq")
                kv_ps = psKV.tile([128, KVW], F32, tag="akv", name="akv")
                if it in xs_hoist:
                    xstrip = xs_hoist.pop(it)
                else:
                    xstrip = xpool.tile([128, NE * 128], BF16, tag="xstrip",
                                        name="xstrip")
                    nc.sync.dma_start(out=xstrip, in_=xT_d[it * 128:(it + 1) * 128, :])
                for e in range(NE):
                    xt = xstrip[:, e * 128:(e + 1) * 128]
                    last = zero_bias and e == NE - 1
                    nc.tensor.matmul(kv_ps, xt, wqkv_sb[e][:, QW:WW],
                                     start=(e == 0), stop=last)
                for e in range(NE):
                    xt = xstrip[:, e * 128:(e + 1) * 128]
                    last = zero_bias and e == NE - 1
                    nc.tensor.matmul(q_ps, xt, wqkv_sb[e][:, 0:QW],
                                     start=(e == 0), stop=last)
                if not zero_bias:
                    # bias add fused into the accumulation as a K=1 matmul
                    nc.tensor.matmul(q_ps, ones1, brow[:, 0:QW],
                                     start=False, stop=True)
                    nc.tensor.matmul(kv_ps, ones1, brow[:, QW:WW],
                                     start=False, stop=True)
                if idx == NT - 1:
                    # pre-fill the attention pipeline; si=0 needs only
                    # qT/kT t-tiles 0-1, ready long ago
                    for j in range(LOOKAHEAD):
                        issue_s_mask_exp(j)

                if it in cs_hoist:
                    cq, sq, ck, sk = cs_hoist.pop(it)
                else:
                    cq = cspool.tile([128, 128], F32, tag="cq", name="cq")
                    nc.gpsimd.dma_start(out=cq, in_=cq_d[it * 128:(it + 1) * 128, :])
                    sq = cspool.tile([128, 128], F32, tag="sq", name="sq")
                    nc.gpsimd.dma_start(out=sq, in_=sq_d[it * 128:(it + 1) * 128, :])
                    ck = cspool.tile([128, 128], F32, tag="ck", name="ck")
                    nc.gpsimd.dma_start(out=ck, in_=ck_d[it * 128:(it + 1) * 128, :])
                    sk = cspool.tile([128, 128], F32, tag="sk", name="sk")
                    nc.gpsimd.dma_start(out=sk, in_=sk_d[it * 128:(it + 1) * 128, :])
                if idx == 2:
                    nc.sync.dma_start(out=mask_sb, in_=mask_d[:, :])
                    for hh in range(HP):
                        nc.sync.dma_start(
                            out=wo_sb[hh], in_=wo_d[hh * 128:(hh + 1) * 128, :])

                if idx == NT - 1:
                    # free the A psum banks early: the B pools allocate them
                    qsb = scratch.tile([128, QW], F32, tag="qsb", name="qsb")
                    nc.scalar.copy(out=qsb, in_=q_ps)
                    kvsb = scratch.tile([128, KVW], F32, tag="kvsb", name="kvsb")
                    nc.scalar.copy(out=kvsb, in_=kv_ps)
                    q_src, kv_src = qsb, kvsb
                else:
                    q_src, kv_src = q_ps, kv_ps
                nc.vector.tensor_scalar_mul(
                    vone[it][:, 0:128], kv_src[:, 128:256], 1.0)
                for b in (HP, 0, 1, 2, 3):  # k first, then q heads
                    if b < HP:
                        src = q_src[:, b * 128:(b + 1) * 128]
                        c_t, s_t = cq[:, :], sq[:, :]
                    else:
                        src = kv_src[:, 0:128]
                        c_t, s_t = ck[:, :], sk[:, :]
                    sqout = scratch.tile([128, 128], F32, tag="sqout", name="sqout")
                    sqacc = small.tile([128, 1], F32, tag="sqacc", name="sqacc")
                    nc.scalar.activation(
                        out=sqout, in_=src,
                        func=mybir.ActivationFunctionType.Square,
                        accum_out=sqacc,
                    )
                    rstd = small.tile([128, 1], F32, tag="rstd", name="rstd")
                    nc.scalar.activation(
                        out=rstd, in_=sqacc,
                        func=mybir.ActivationFunctionType.Sqrt,
                        bias=eps_t, scale=1.0 / D,
                    )
                    nc.vector.reciprocal(rstd, rstd)
                    # rope: out1 = x1*c1 - x2*s1 ; out2 = x2*c2 + x1*s2
                    rt = scratch.tile([128, 128], F32, tag="rt", name="rt")
                    m1 = scratch.tile([128, 64], F32, tag="m1", name="m1")
                    nc.vector.tensor_mul(rt[:, 0:64], src[:, 0:64], c_t[:, 0:64])
                    nc.vector.tensor_mul(m1, src[:, 64:128], s_t[:, 0:64])
                    nc.vector.tensor_sub(rt[:, 0:64], rt[:, 0:64], m1)
                    m2 = scratch.tile([128, 64], F32, tag="m2", name="m2")
                    nc.vector.tensor_mul(rt[:, 64:128], src[:, 64:128], c_t[:, 64:128])
                    nc.vector.tensor_mul(m2, src[:, 0:64], s_t[:, 64:128])
                    nc.vector.tensor_add(rt[:, 64:128], rt[:, 64:128], m2)
                    rb = scratch.tile([128, 128], BF16, tag="rb", name="rb")
                    nc.vector.tensor_scalar_mul(rb, rt, rstd)
                    tp = psT.tile([128, 128], BF16, tag="tp", name="tp")
                    nc.tensor.transpose(tp, rb, ident)
                    dst = qT[b] if b < HP else kT
                    nc.scalar.copy(out=dst[:, it * 128:(it + 1) * 128], in_=tp)

        phase_a()
        psT_cm.__exit__(None, None, None)
        psKV_cm.__exit__(None, None, None)
        psA_cm.__exit__(None, None, None)

        # ---------------- Phases B+C fused, si-major ------------------------
        # Each open psum accumulation group needs its own bank (zero region):
        # ctx q-blocks live in separate single-buffered pools.
        with tc.tile_pool(name="psC0", bufs=1, space="PSUM") as psC0, \
             tc.tile_pool(name="psC1", bufs=1, space="PSUM") as psC1, \
             tc.tile_pool(name="psT2", bufs=1, space="PSUM") as psT2, \
             tc.tile_pool(name="psO", bufs=2, space="PSUM") as psO:

            cwork = []

            def issue_ctx(idx):
                h, si, g = glist[idx]
                p_g = pend_p.pop(idx)
                if g == 0:
                    c0 = psC0.tile([128, 129], F32, tag="ctxq0", name="ctxq0")
                    c1 = psC1.tile([128, 129], F32, tag="ctxq1", name="ctxq1")
                    ctx_tiles[(h, si)] = (c0, c1)
                cts = ctx_tiles[(h, si)]
                for half in range(2):
                    jb = 2 * g + half
                    kk = jb - 2 * si
                    for qb in range(2):
                        if kk > qb:
                            continue  # fully-masked subblock: skip
                        nc.tensor.matmul(
                            cts[qb],
                            p_g[:, half * 256 + qb * 128:half * 256 + (qb + 1) * 128],
                            vone[jb],
                            start=(jb == 0), stop=(jb == 2 * si + qb),
                        )

            def normalize(h, si):
                cts = ctx_tiles.pop((h, si))
                for qb in range(2):
                    ct = cts[qb]
                    rden = small.tile([128, 1], F32, tag="rden", name="rden")
                    nc.vector.reciprocal(rden, ct[:, 128:129])
                    cbs = scratch.tile([128, 128], BF16, tag="cbs", name="cbs")
                    nc.vector.tensor_scalar_mul(cbs, ct[:, 0:128], rden)
                    tp = psT2.tile([128, 128], BF16, tag="tp2", name="tp2")
                    nc.tensor.transpose(tp, cbs, ident)
                    nc.vector.tensor_scalar_mul(
                        ctxT[h][:, si * 256 + qb * 128:si * 256 + (qb + 1) * 128],
                        tp, 1.0)

            def c_group(it, ot):
                o_ps = psO.tile([128, OC], F32, tag="ops", name="ops")
                for h in range(HP):
                    nc.tensor.matmul(
                        o_ps,
                        ctxT[h][:, it * 128:(it + 1) * 128],
                        wo_sb[h][:, ot * OC:(ot + 1) * OC],
                        start=(h == 0), stop=(h == HP - 1),
                    )
                o_sb = opool.tile([128, OC], F32, tag="osb", name="osb")
                hw = OC // 2
                nc.vector.tensor_scalar_mul(o_sb[:, 0:hw], o_ps[:, 0:hw], 1.0)
                nc.scalar.copy(out=o_sb[:, hw:OC], in_=o_ps[:, hw:OC])
                eng = nc.sync if (it + ot) % 2 == 0 else nc.gpsimd
                eng.dma_start(
                    out=out_d[it * 128:(it + 1) * 128, ot * OC:(ot + 1) * OC],
                    in_=o_sb)

            for i, (h, si, g) in enumerate(glist):
                if i + LOOKAHEAD < len(glist):
                    issue_s_mask_exp(i + LOOKAHEAD)
                for _ in range(3):  # drain out-projection work into PE slack
                    if cwork:
                        c_group(*cwork.pop(0))
                issue_ctx(i)
                if g == si:  # last group of (h, si)
                    normalize(h, si)
                    if h == HP - 1:
                        cwork.extend(
                            (it, ot)
                            for it in range(2 * si, 2 * si + 2)
                            for ot in range(NO))
            while cwork:
                c_group(*cwork.pop(0))
        psS2_cm.__exit__(None, None, None)
        psS_cm.__exit__(None, None, None)

    return nc


def _prep_inputs(x, mask, cos, sin, wq, bq, wk, bk, wv, bv, wo, q_scale, k_scale):
    x2 = np.asarray(x, dtype=np.float32).reshape(T, EMB)
    # strip layout: row (it*128 + p), col (eb*128 + t) holds x[it*128+t, eb*128+p]
    xTt = x2.reshape(NT, 128, NE, 128).transpose(0, 3, 2, 1)
    xTt = np.ascontiguousarray(xTt).reshape(NT * 128, NE * 128).astype(BF)

    qs = np.asarray(q_scale, dtype=np.float32)
    ks = np.asarray(k_scale, dtype=np.float32)
    qs_rot = np.concatenate([qs[64:], qs[:64]])
    ks_rot = np.concatenate([ks[64:], ks[:64]])
    cos = np.asarray(cos, dtype=np.float32)
    sin = np.asarray(sin, dtype=np.float32)
    cosq = np.ascontiguousarray(cos * qs[None, :])
    sinq = np.ascontiguousarray(sin * qs_rot[None, :])
    cosk = np.ascontiguousarray(cos * ks[None, :])
    sink = np.ascontiguousarray(sin * ks_rot[None, :])

    jj = np.arange(128)[:, None]
    cc = np.arange(128)[None, :]
    tri = np.where(jj > cc, NEG, 0.0).astype(np.float32)
    z = np.zeros((128, 128), np.float32)
    mask256 = np.concatenate([tri, z, z, tri], axis=1)

    wq = np.asarray(wq, dtype=np.float32)
    wk = np.asarray(wk, dtype=np.float32)
    wv = np.asarray(wv, dtype=np.float32)
    wo = np.asarray(wo, dtype=np.float32)
    bq = np.asarray(bq, dtype=np.float32)
    bk = np.asarray(bk, dtype=np.float32)
    bv = np.asarray(bv, dtype=np.float32)

    in_maps = []
    for c in range(NCORES):
        wqkv_c = np.ascontiguousarray(
            np.concatenate([wq[:, c * QW:(c + 1) * QW],
                            wk[:, c * D:(c + 1) * D],
                            wv[:, c * D:(c + 1) * D]], axis=1)).astype(BF)
        wo_c = np.ascontiguousarray(wo[c * QW:(c + 1) * QW, :]).astype(BF)
        brow_c = np.concatenate(
            [bq[c * QW:(c + 1) * QW], bk[c * D:(c + 1) * D],
             bv[c * D:(c + 1) * D]]).reshape(1, WW).astype(BF)
        in_maps.append({
            "xT": xTt,
            "wqkv": wqkv_c,
            "wo": wo_c,
            "cosq": cosq, "sinq": sinq, "cosk": cosk, "sink": sink,
            "mask256": mask256,
            "ident": np.eye(128, dtype=BF),
            "brow": brow_c,
        })
    return in_maps


def _get_program(zero_bias=True):
    key = ("nc", zero_bias)
    if key not in _prog_cache:
        nc = _build_program(zero_bias)
        if not nc.is_finalized():
            nc.finalize()
        _prog_cache[key] = nc
    return _prog_cache[key]


def kernel(**inputs):
    zb = not (np.any(inputs["bq"]) or np.any(inputs["bk"])
              or np.any(inputs["bv"]))
    in_maps = _prep_inputs(**inputs)
    nc = _get_program(bool(zb))
    res = run_bass_kernel_spmd(nc, in_maps, list(range(NCORES)))
    out = np.zeros((T, EMB), dtype=np.float32)
    for r in res.results:
        out += np.asarray(r["out"], dtype=np.float32)
    return out.reshape(1, T, EMB)


# revision 12
# speedup vs baseline: 1.0325x; 1.0002x over previous
"""GQA attention kernel for Trainium2, tensor-parallel over heads on 8 cores.

Problem: B=1, T=2048, EMB=4096, H=32 query heads, G=8 KV groups, D=128.
Reference: q/k/v projections -> per-head RMS norm (q,k) -> RoPE (q,k) ->
causal GQA attention -> out projection.

Sharding: core c owns query heads [4c, 4c+4) and KV group c.  Each core
computes a partial output x @ W_o_rows for its heads; host sums the 8
partials (the all-reduce of the module's TP scheme).

Design notes:
  - softmax denominator is FREE: ctx matmuls use P-block [k,128q] as the
    stationary operand and [v | ones] (129 moving cols) as the moving one;
    den accumulates in psum col 128 of the same accumulation group.
  - normalization is a per-partition tensor_scalar multiply (q is the
    partition dim of the ctx psum tile) -- no partition broadcast needed.
  - attention runs in 256-query slices; causal sub-block skipping: the
    ctx matmul (qb, jb) is only issued when jb <= 2*si+qb, so the
    fully-masked P regions are never read; the mask is a [128,128]
    triangle added only on diagonal subblocks (post-matmul on DVE --
    PE accumulate does NOT see DVE-prewritten psum on real HW).
  - qkv biases fold into the projection accumulation as a K=1 matmul
    (ones[1,128] x brow[1,768]); when the biases are all zero (the
    common case) a specialized program variant skips them entirely.
  - each open psum accumulation group needs its own bank (zero region):
    8 banks = psS(2)+psS2(1) S-groups, ctxq0(1), ctxq1(1), psT2(1),
    psO(2); S-groups rotate 3-deep across psS/psS2.
  - phases: A (projections+rms+rope+transpose), then B (attention) and
    C (out projection) fused: C ot-groups are drained into the B stream
    (2 per group-iteration) so PE fills ACT-bound exp slack; the first
    B S/exp groups are issued during the last A tile.
  - the SP queue issues DMAs serially (blocking for the transfer), so
    DMA program order == arrival order: wqkv e0-3, x strip 0, e4-15,
    x strip 1, rope tables, consts, remaining wqkv; wo/mask issued at
    A-tile 2; exp table loaded once (Square/Copy live in every set).
    The last A tile copies q/kv psum to SBUF so the A psum banks (and
    with them the B pools) free before its rms/rope chain drains.
"""

import numpy as np
import ml_dtypes
from contextlib import ExitStack

import concourse.bacc as bacc
import concourse.mybir as mybir
from concourse.tile import TileContext
from concourse.bass_utils import run_bass_kernel_spmd

EMB, H, G, D, T = 4096, 32, 8, 128, 2048
EPS = 1e-6
NCORES = 8
HP = H // NCORES          # 4 query heads per core
NT = T // 128             # 16 t-tiles
NE = EMB // 128           # 32 e-tiles
QW = HP * D               # 512 = q width per core
KVW = 2 * D               # 256 = k|v width per core
WW = QW + KVW             # 768 = packed qkv width per core
SM_SCALE = 1.0 / float(np.sqrt(D))
NEG = -1e9
OC = 512                  # phase C output column tile width
NO = EMB // OC            # 16 output column tiles

F32 = mybir.dt.float32
BF16 = mybir.dt.bfloat16
BF = ml_dtypes.bfloat16

_prog_cache = {}


def _build_program(zero_bias):
    nc = bacc.Bacc()

    xT_d = nc.declare_dram_parameter("xT", [NT * 128, NE * 128], BF16, isOutput=False)
    wqkv_d = nc.declare_dram_parameter("wqkv", [NE * 128, WW], BF16, isOutput=False)
    wo_d = nc.declare_dram_parameter("wo", [HP * 128, EMB], BF16, isOutput=False)
    cq_d = nc.declare_dram_parameter("cosq", [NT * 128, 128], F32, isOutput=False)
    sq_d = nc.declare_dram_parameter("sinq", [NT * 128, 128], F32, isOutput=False)
    ck_d = nc.declare_dram_parameter("cosk", [NT * 128, 128], F32, isOutput=False)
    sk_d = nc.declare_dram_parameter("sink", [NT * 128, 128], F32, isOutput=False)
    mask_d = nc.declare_dram_parameter("mask256", [128, 512], F32, isOutput=False)
    ident_d = nc.declare_dram_parameter("ident", [128, 128], BF16, isOutput=False)
    brow_d = (None if zero_bias else
              nc.declare_dram_parameter("brow", [1, WW], BF16, isOutput=False))
    out_d = nc.declare_dram_parameter("out", [T, EMB], F32, isOutput=True)

    with TileContext(nc) as tc, ExitStack() as ctx:
        consts = ctx.enter_context(tc.tile_pool(name="consts", bufs=1))
        wpool = ctx.enter_context(tc.tile_pool(name="wpool", bufs=1))
        xpool = ctx.enter_context(tc.tile_pool(name="xpool", bufs=2))
        cspool = ctx.enter_context(tc.tile_pool(name="cspool", bufs=2))
        scratch = ctx.enter_context(tc.tile_pool(name="scratch", bufs=8))
        small = ctx.enter_context(tc.tile_pool(name="small", bufs=12))
        resid = ctx.enter_context(tc.tile_pool(name="resid", bufs=1))
        ppool = ctx.enter_context(tc.tile_pool(name="ppool", bufs=6))
        opool = ctx.enter_context(tc.tile_pool(name="opool", bufs=6))

        # ---- early DMAs: the SP queue issues serially, so order = arrival.
        # First-needed first: wqkv e=0-3, x strip 0, more wqkv, x strip 1,
        # rope tables for tiles 0-1, brow/ident, remaining wqkv strips.
        TILE_ORDER = list(range(NT))
        wqkv_sb = [wpool.tile([128, WW], BF16, tag=f"wqkv{e}", name=f"wqkv{e}")
                   for e in range(NE)]

        def _wq_dma(e):
            nc.sync.dma_start(out=wqkv_sb[e],
                              in_=wqkv_d[e * 128:(e + 1) * 128, :])

        for e in range(4):
            _wq_dma(e)
        # hoisted strips go out on the otherwise-idle ACT DMA queue, rope
        # tables + ident on the DVE queue -- all three queues transfer in
        # parallel with the SP weight stream during startup
        xs_hoist = {}
        for hidx, it in enumerate(TILE_ORDER[:2]):
            xs = xpool.tile([128, NE * 128], BF16, tag="xstrip", name="xstrip")
            nc.scalar.dma_start(out=xs, in_=xT_d[it * 128:(it + 1) * 128, :])
            xs_hoist[it] = xs
        cs_hoist = {}
        for it in TILE_ORDER[:2]:
            tiles = []
            for nm, d in (("cq", cq_d), ("sq", sq_d), ("ck", ck_d), ("sk", sk_d)):
                t = cspool.tile([128, 128], F32, tag=nm, name=nm)
                nc.gpsimd.dma_start(out=t, in_=d[it * 128:(it + 1) * 128, :])
                tiles.append(t)
            cs_hoist[it] = tiles
        ident = consts.tile([128, 128], BF16, tag="ident", name="ident")
        nc.gpsimd.dma_start(out=ident, in_=ident_d[:, :])
        for e in range(4, 11):
            _wq_dma(e)
        eps_t = consts.tile([128, 1], F32, tag="eps", name="eps")
        nc.vector.memset(eps_t, EPS)
        # pre-warm the Sqrt act-func set during the DMA dead time so the
        # 1.3us table loads stay off the first tile's rms chain (Square and
        # Copy live in the same set; only Exp needs another load, at B start)
        warm_t = consts.tile([128, 1], F32, tag="warm", name="warm")
        nc.scalar.activation(out=warm_t, in_=eps_t,
                             func=mybir.ActivationFunctionType.Sqrt)
        # PE p-state warm-up: keep the PE continuously busy on dummy matmuls
        # through the initial DMA wait so the clock is fully ramped when the
        # first projection matmul's inputs land. A late ramp costs ~2.5us;
        # if the DMAs beat the dummies the loss is bounded by one dummy.
        dmA = consts.tile([128, 128], BF16, tag="dmA", name="dmA")
        nc.vector.memset(dmA, 0.0)
        dmB = consts.tile([128, 512], BF16, tag="dmB", name="dmB")
        nc.vector.memset(dmB, 0.0)
        if not zero_bias:
            brow = consts.tile([1, WW], BF16, tag="brow", name="brow")
            nc.sync.dma_start(out=brow, in_=brow_d[:, :])
            ones1 = consts.tile([1, 128], BF16, tag="ones1", name="ones1")
            nc.vector.memset(ones1, 1.0)
        for e in range(11, NE):
            _wq_dma(e)
        mask_sb = consts.tile([128, 512], F32, tag="mask", name="mask")
        wo_sb = [wpool.tile([128, EMB], BF16, tag=f"wo{h}", name=f"wo{h}")
                 for h in range(HP)]

        # resident activations
        qT = [resid.tile([128, T], BF16, tag=f"qT{h}", name=f"qT{h}") for h in range(HP)]
        kT = resid.tile([128, T], BF16, tag="kT", name="kT")
        vone = [resid.tile([128, 129], BF16, tag=f"v{j}", name=f"v{j}") for j in range(NT)]
        ctxT = [resid.tile([128, T], BF16, tag=f"ctxT{h}", name=f"ctxT{h}") for h in range(HP)]
        for j in range(NT):
            nc.vector.memset(vone[j][:, 128:129], 1.0)

        # ---------------- Phases ------------------------------------------
        # A: projections + rms + rope + transpose.  B: attention (256-query
        # slices, P-stationary ctx matmuls with a ones column for the free
        # softmax denominator).  C: out projection, interleaved into B's
        # stream so PE fills ACT-bound slack.  The first B groups are issued
        # during late phase A (psS opens alongside the A pools).
        psS_cm = tc.tile_pool(name="psS", bufs=2, space="PSUM")
        psS2_cm = tc.tile_pool(name="psS2", bufs=1, space="PSUM")
        psA_cm = tc.tile_pool(name="psA", bufs=2, space="PSUM")
        psKV_cm = tc.tile_pool(name="psKV", bufs=2, space="PSUM")
        psT_cm = tc.tile_pool(name="psT", bufs=1, space="PSUM")
        psS = psS_cm.__enter__()
        psS2_pool_early = psS2_cm.__enter__()
        psA = psA_cm.__enter__()
        psKV = psKV_cm.__enter__()
        psT = psT_cm.__enter__()

        NSI = 8
        glist = []
        for si in range(NSI):
            for h in range(HP):
                glist.extend((h, si, g) for g in range(si + 1))
        pend_p = {}
        ctx_tiles = {}
        psS2 = [None]
        psS2[0] = psS2_pool_early

        def issue_s_mask_exp(idx):
            h, si, g = glist[idx]
            pool = psS if idx % 3 < 2 else psS2[0]
            s_g = pool.tile([128, 512], F32, tag="sg", name="sg")
            for half in range(2):
                jb = 2 * g + half
                nc.tensor.matmul(
                    s_g[:, half * 256:(half + 1) * 256],
                    kT[:, jb * 128:(jb + 1) * 128],
                    qT[h][:, si * 256:(si + 1) * 256],
                    start=True, stop=True,
                )
            for half in range(2):
                kk = 2 * g + half - 2 * si
                if kk >= 0:  # diagonal: triangular mask on its subblock
                    off = half * 256 + kk * 128
                    nc.vector.tensor_add(
                        s_g[:, off:off + 128], s_g[:, off:off + 128],
                        mask_sb[:, kk * 256 + kk * 128:kk * 256 + (kk + 1) * 128])
            p_g = ppool.tile([128, 512], BF16, tag="pt", name="pt")
            nc.scalar.activation(
                out=p_g, in_=s_g,
                func=mybir.ActivationFunctionType.Exp,
                scale=SM_SCALE,
            )
            pend_p[idx] = p_g

        LOOKAHEAD = 3

        def phase_a():
            for w in range(13):
                wps = psS.tile([128, 512], F32, tag="sg", name="sg")
                nc.tensor.matmul(wps, dmA, dmB, start=True, stop=True)
            for idx, it in enumerate(TILE_ORDER):
                q_ps = psA.tile([128, QW], F32, tag="aq", name="aq")
                kv_ps = psKV.tile([128, KVW], F32, tag="akv", name="akv")
                if it in xs_hoist:
                    xstrip = xs_hoist.pop(it)
                else:
                    xstrip = xpool.tile([128, NE * 128], BF16, tag="xstrip",
                                        name="xstrip")
                    nc.sync.dma_start(out=xstrip, in_=xT_d[it * 128:(it + 1) * 128, :])
                for e in range(NE):
                    xt = xstrip[:, e * 128:(e + 1) * 128]
                    last = zero_bias and e == NE - 1
                    nc.tensor.matmul(kv_ps, xt, wqkv_sb[e][:, QW:WW],
                                     start=(e == 0), stop=last)
                for e in range(NE):
                    xt = xstrip[:, e * 128:(e + 1) * 128]
                    last = zero_bias and e == NE - 1
                    nc.tensor.matmul(q_ps, xt, wqkv_sb[e][:, 0:QW],
                                     start=(e == 0), stop=last)
                if not zero_bias:
                    # bias add fused into the accumulation as a K=1 matmul
                    nc.tensor.matmul(q_ps, ones1, brow[:, 0:QW],
                                     start=False, stop=True)
                    nc.tensor.matmul(kv_ps, ones1, brow[:, QW:WW],
                                     start=False, stop=True)
                if idx == NT - 1:
                    # pre-fill the attention pipeline; si=0 needs only
                    # qT/kT t-tiles 0-1, ready long ago
                    for j in range(LOOKAHEAD):
                        issue_s_mask_exp(j)

                if it in cs_hoist:
                    cq, sq, ck, sk = cs_hoist.pop(it)
                else:
                    cq = cspool.tile([128, 128], F32, tag="cq", name="cq")
                    nc.gpsimd.dma_start(out=cq, in_=cq_d[it * 128:(it + 1) * 128, :])
                    sq = cspool.tile([128, 128], F32, tag="sq", name="sq")
                    nc.gpsimd.dma_start(out=sq, in_=sq_d[it * 128:(it + 1) * 128, :])
                    ck = cspool.tile([128, 128], F32, tag="ck", name="ck")
                    nc.gpsimd.dma_start(out=ck, in_=ck_d[it * 128:(it + 1) * 128, :])
                    sk = cspool.tile([128, 128], F32, tag="sk", name="sk")
                    nc.gpsimd.dma_start(out=sk, in_=sk_d[it * 128:(it + 1) * 128, :])
                if idx == 2:
                    nc.sync.dma_start(out=mask_sb, in_=mask_d[:, :])
                    for hh in range(HP):
                        nc.sync.dma_start(
                            out=wo_sb[hh], in_=wo_d[hh * 128:(hh + 1) * 128, :])

                if idx == NT - 1:
                    # free the A psum banks early: the B pools allocate them
                    qsb = scratch.tile([128, QW], F32, tag="qsb", name="qsb")
                    nc.scalar.copy(out=qsb, in_=q_ps)
                    kvsb = scratch.tile([128, KVW], F32, tag="kvsb", name="kvsb")
                    nc.scalar.copy(out=kvsb, in_=kv_ps)
                    q_src, kv_src = qsb, kvsb
                else:
                    q_src, kv_src = q_ps, kv_ps
                nc.vector.tensor_scalar_mul(
                    vone[it][:, 0:128], kv_src[:, 128:256], 1.0)
                for b in (HP, 0, 1, 2, 3):  # k first, then q heads
                    if b < HP:
                        src = q_src[:, b * 128:(b + 1) * 128]
                        c_t, s_t = cq[:, :], sq[:, :]
                    else:
                        src = kv_src[:, 0:128]
                        c_t, s_t = ck[:, :], sk[:, :]
                    sqout = scratch.tile([128, 128], F32, tag="sqout", name="sqout")
                    sqacc = small.tile([128, 1], F32, tag="sqacc", name="sqacc")
                    nc.scalar.activation(
                        out=sqout, in_=src,
                        func=mybir.ActivationFunctionType.Square,
                        accum_out=sqacc,
                    )
                    rstd = small.tile([128, 1], F32, tag="rstd", name="rstd")
                    nc.scalar.activation(
                        out=rstd, in_=sqacc,
                        func=mybir.ActivationFunctionType.Sqrt,
                        bias=eps_t, scale=1.0 / D,
                    )
                    nc.vector.reciprocal(rstd, rstd)
                    # rope: out1 = x1*c1 - x2*s1 ; out2 = x2*c2 + x1*s2
                    rt = scratch.tile([128, 128], F32, tag="rt", name="rt")
                    m1 = scratch.tile([128, 64], F32, tag="m1", name="m1")
                    nc.vector.tensor_mul(rt[:, 0:64], src[:, 0:64], c_t[:, 0:64])
                    nc.vector.tensor_mul(m1, src[:, 64:128], s_t[:, 0:64])
                    nc.vector.tensor_sub(rt[:, 0:64], rt[:, 0:64], m1)
                    m2 = scratch.tile([128, 64], F32, tag="m2", name="m2")
                    nc.vector.tensor_mul(rt[:, 64:128], src[:, 64:128], c_t[:, 64:128])
                    nc.vector.tensor_mul(m2, src[:, 0:64], s_t[:, 64:128])
                    nc.vector.tensor_add(rt[:, 64:128], rt[:, 64:128], m2)
                    rb = scratch.tile([128, 128], BF16, tag="rb", name="rb")
                    nc.vector.tensor_scalar_mul(rb, rt, rstd)
                    tp = psT.tile([128, 128], BF16, tag="tp", name="tp")
                    nc.tensor.transpose(tp, rb, ident)
                    dst = qT[b] if b < HP else kT
                    nc.scalar.copy(out=dst[:, it * 128:(it + 1) * 128], in_=tp)

        phase_a()
        psT_cm.__exit__(None, None, None)
        psKV_cm.__exit__(None, None, None)
        psA_cm.__exit__(None, None, None)

        # ---------------- Phases B+C fused, si-major ------------------------
        # Each open psum accumulation group needs its own bank (zero region):
        # ctx q-blocks live in separate single-buffered pools.
        with tc.tile_pool(name="psC0", bufs=1, space="PSUM") as psC0, \
             tc.tile_pool(name="psC1", bufs=1, space="PSUM") as psC1, \
             tc.tile_pool(name="psT2", bufs=1, space="PSUM") as psT2, \
             tc.tile_pool(name="psO", bufs=2, space="PSUM") as psO:

            cwork = []

            def issue_ctx(idx):
                h, si, g = glist[idx]
                p_g = pend_p.pop(idx)
                if g == 0:
                    c0 = psC0.tile([128, 129], F32, tag="ctxq0", name="ctxq0")
                    c1 = psC1.tile([128, 129], F32, tag="ctxq1", name="ctxq1")
                    ctx_tiles[(h, si)] = (c0, c1)
                cts = ctx_tiles[(h, si)]
                for half in range(2):
                    jb = 2 * g + half
                    kk = jb - 2 * si
                    for qb in range(2):
                        if kk > qb:
                            continue  # fully-masked subblock: skip
                        nc.tensor.matmul(
                            cts[qb],
                            p_g[:, half * 256 + qb * 128:half * 256 + (qb + 1) * 128],
                            vone[jb],
                            start=(jb == 0), stop=(jb == 2 * si + qb),
                        )

            def normalize(h, si):
                cts = ctx_tiles.pop((h, si))
                for qb in range(2):
                    ct = cts[qb]
                    rden = small.tile([128, 1], F32, tag="rden", name="rden")
                    nc.vector.reciprocal(rden, ct[:, 128:129])
                    cbs = scratch.tile([128, 128], BF16, tag="cbs", name="cbs")
                    nc.vector.tensor_scalar_mul(cbs, ct[:, 0:128], rden)
                    tp = psT2.tile([128, 128], BF16, tag="tp2", name="tp2")
                    nc.tensor.transpose(tp, cbs, ident)
                    nc.vector.tensor_scalar_mul(
                        ctxT[h][:, si * 256 + qb * 128:si * 256 + (qb + 1) * 128],
                        tp, 1.0)

            def c_group(it, ot):
                o_ps = psO.tile([128, OC], F32, tag="ops", name="ops")
                for h in range(HP):
                    nc.tensor.matmul(
                        o_ps,
                        ctxT[h][:, it * 128:(it + 1) * 128],
                        wo_sb[h][:, ot * OC:(ot + 1) * OC],
                        start=(h == 0), stop=(h == HP - 1),
                    )
                o_sb = opool.tile([128, OC], F32, tag="osb", name="osb")
                hw = OC // 2
                nc.vector.tensor_scalar_mul(o_sb[:, 0:hw], o_ps[:, 0:hw], 1.0)
                nc.scalar.copy(out=o_sb[:, hw:OC], in_=o_ps[:, hw:OC])
                eng = nc.sync if (it + ot) % 2 == 0 else nc.gpsimd
                eng.dma_start(
                    out=out_d[it * 128:(it + 1) * 128, ot * OC:(ot + 1) * OC],
                    in_=o_sb)

            for i, (h, si, g) in enumerate(glist):
                if i + LOOKAHEAD < len(glist):
                    issue_s_mask_exp(i + LOOKAHEAD)
                for _ in range(3):  # drain out-projection work into PE slack
                    if cwork:
                        c_group(*cwork.pop(0))
                issue_ctx(i)
                if g == si:  # last group of (h, si)
                    normalize(h, si)
                    if h == HP - 1:
                        cwork.extend(
                            (it, ot)
                            for it in range(2 * si, 2 * si + 2)
                            for ot in range(NO))
            while cwork:
                c_group(*cwork.pop(0))
        psS2_cm.__exit__(None, None, None)
        psS_cm.__exit__(None, None, None)

    return nc


def _prep_inputs(x, mask, cos, sin, wq, bq, wk, bk, wv, bv, wo, q_scale, k_scale):
    x2 = np.asarray(x, dtype=np.float32).reshape(T, EMB)
    # strip layout: row (it*128 + p), col (eb*128 + t) holds x[it*128+t, eb*128+p]
    xTt = x2.reshape(NT, 128, NE, 128).transpose(0, 3, 2, 1)
    xTt = np.ascontiguousarray(xTt).reshape(NT * 128, NE * 128).astype(BF)

    qs = np.asarray(q_scale, dtype=np.float32)
    ks = np.asarray(k_scale, dtype=np.float32)
    qs_rot = np.concatenate([qs[64:], qs[:64]])
    ks_rot = np.concatenate([ks[64:], ks[:64]])
    cos = np.asarray(cos, dtype=np.float32)
    sin = np.asarray(sin, dtype=np.float32)
    cosq = np.ascontiguousarray(cos * qs[None, :])
    sinq = np.ascontiguousarray(sin * qs_rot[None, :])
    cosk = np.ascontiguousarray(cos * ks[None, :])
    sink = np.ascontiguousarray(sin * ks_rot[None, :])

    jj = np.arange(128)[:, None]
    cc = np.arange(128)[None, :]
    tri = np.where(jj > cc, NEG, 0.0).astype(np.float32)
    z = np.zeros((128, 128), np.float32)
    mask256 = np.concatenate([tri, z, z, tri], axis=1)

    wq = np.asarray(wq, dtype=np.float32)
    wk = np.asarray(wk, dtype=np.float32)
    wv = np.asarray(wv, dtype=np.float32)
    wo = np.asarray(wo, dtype=np.float32)
    bq = np.asarray(bq, dtype=np.float32)
    bk = np.asarray(bk, dtype=np.float32)
    bv = np.asarray(bv, dtype=np.float32)

    in_maps = []
    for c in range(NCORES):
        wqkv_c = np.ascontiguousarray(
            np.concatenate([wq[:, c * QW:(c + 1) * QW],
                            wk[:, c * D:(c + 1) * D],
                            wv[:, c * D:(c + 1) * D]], axis=1)).astype(BF)
        wo_c = np.ascontiguousarray(wo[c * QW:(c + 1) * QW, :]).astype(BF)
        brow_c = np.concatenate(
            [bq[c * QW:(c + 1) * QW], bk[c * D:(c + 1) * D],
             bv[c * D:(c + 1) * D]]).reshape(1, WW).astype(BF)
        in_maps.append({
            "xT": xTt,
            "wqkv": wqkv_c,
            "wo": wo_c,
            "cosq": cosq, "sinq": sinq, "cosk": cosk, "sink": sink,
            "mask256": mask256,
            "ident": np.eye(128, dtype=BF),
            "brow": brow_c,
        })
    return in_maps


def _get_program(zero_bias=True):
    key = ("nc", zero_bias)
    if key not in _prog_cache:
        nc = _build_program(zero_bias)
        if not nc.is_finalized():
            nc.finalize()
        _prog_cache[key] = nc
    return _prog_cache[key]


def kernel(**inputs):
    zb = not (np.any(inputs["bq"]) or np.any(inputs["bk"])
              or np.any(inputs["bv"]))
    in_maps = _prep_inputs(**inputs)
    nc = _get_program(bool(zb))
    res = run_bass_kernel_spmd(nc, in_maps, list(range(NCORES)))
    out = np.zeros((T, EMB), dtype=np.float32)
    for r in res.results:
        out += np.asarray(r["out"], dtype=np.float32)
    return out.reshape(1, T, EMB)
